# revision 1
# baseline (speedup 1.0000x reference)
"""Trainium2 Bass kernel for CrossAttention + GroupNorm + Swish (nn_CrossAttention).

Reference computation (per batch element b, xf = x[b] reshaped [C, N]):
    q  = Wq @ xf + bq                       [C, N]
    k  = Wk @ ctx^T + bk                    [C, L]
    v  = Wv @ ctx^T + bv                    [C, L]
    qk = (q^T k) * C^-0.5                   [N, L]
    w  = softmax(qk, axis=-1)
    h  = v @ w^T                            [C, N]
    o  = Wo @ h + bo
    xr = o + xf
    out = swish(groupnorm(xr; 32 groups over (C/32, N)) * gamma + beta)

Sharding: data-parallel over batch B=8 across the 8 NeuronCores (no collectives).

Device algorithm (per core):
  - x chunks resident in SBUF ([128|128|64] x 13824 fp32), updated in place with xr.
  - n-tiles of 384 (36 tiles): all matmuls in fp32r (full PE rate for moving dim >= 256).
  - softmax without max-subtraction (scores are tiny: |qk| <~ 2), with the C^-0.5
    scale folded into Wq/bq on the host.
  - colsum of exp-scores + broadcast across partitions in ONE all-ones matmul.
  - GroupNorm stats via bn_stats/bn_aggr per channel + group-membership matmuls
    (gmap [C,32], gmapT [32,C]) for cross-partition group reduction; bias bo is
    folded analytically (mean_c += bo_c; pass-2 affine absorbs a_c*(bo_c - mu_g)).
  - pass 2: out = Silu(a_c * xr + b_c) as a single ACT op per chunk, in place.
"""
import sys

sys.path.insert(0, "/opt/trn_rl_repo")

import numpy as np

import concourse.tile as tile
from concourse import bacc, mybir
from concourse.bass_utils import run_bass_kernel_spmd

F32 = mybir.dt.float32
F32R = mybir.dt.float32r
AF = mybir.ActivationFunctionType
ALU = mybir.AluOpType

# Problem shapes (hardcoded; harness contract)
B, C, D, L, CTX = 8, 320, 24, 77, 768
N = D * D * D            # 13824 spatial positions
G = 32                   # groupnorm groups
GS = C // G              # 10 channels per group
EPS = 1e-5
N_TILE = 256
NT = N // N_TILE         # 36
CCH = [(0, 128), (128, 128), (256, 64)]   # channel chunks (start, size)
KV_COLS = 768            # padded concat [k0,k1,k2+pad, v0,v1,v2+pad]

# Silu isn't implemented in CoreSim; test.py flips this for simulation runs.
USE_SILU = True

# bias6 columns
BQ, BK, BV, BO, GA, BE = range(6)


def _build():
    nc = bacc.Bacc(trn_type="TRN2", target_bir_lowering=False, debug=False)

    x_d = nc.dram_tensor("x", [C, N], F32, kind="ExternalInput")
    ctx_d = nc.dram_tensor("ctx", [L, CTX], F32, kind="ExternalInput")
    wqt_d = nc.dram_tensor("wqt", [C, C], F32R, kind="ExternalInput")
    wot_d = nc.dram_tensor("wot", [C, C], F32R, kind="ExternalInput")
    wkvt_d = nc.dram_tensor("wkvt", [CTX, KV_COLS], F32, kind="ExternalInput")
    bias6_d = nc.dram_tensor("bias6", [C, 6], F32, kind="ExternalInput")
    gmap_d = nc.dram_tensor("gmap", [C, G], F32, kind="ExternalInput")
    gmapt_d = nc.dram_tensor("gmapt", [G, C], F32, kind="ExternalInput")
    ones77_d = nc.dram_tensor("ones77", [L, L], F32R, kind="ExternalInput")
    ident_d = nc.dram_tensor("ident", [128, 128], F32, kind="ExternalInput")
    out_d = nc.dram_tensor("out", [C, N], F32, kind="ExternalOutput")

    with tile.TileContext(nc) as tc:
        _emit(nc, tc, x_d, ctx_d, wqt_d, wot_d, wkvt_d, bias6_d, gmap_d,
              gmapt_d, ones77_d, ident_d, out_d)
    nc.compile()
    return nc


def _emit(nc, tc, x_d, ctx_d, wqt_d, wot_d, wkvt_d, bias6_d, gmap_d,
          gmapt_d, ones77_d, ident_d, out_d):
    from contextlib import ExitStack

    with ExitStack() as ctx_stack:
        const = ctx_stack.enter_context(tc.tile_pool(name="const", bufs=1))
        xpool = ctx_stack.enter_context(tc.tile_pool(name="xbuf", bufs=1))
        kvres = ctx_stack.enter_context(tc.tile_pool(name="kvres", bufs=1))
        psum = ctx_stack.enter_context(tc.tile_pool(name="psum", bufs=8, space="PSUM"))
        loop = ctx_stack.enter_context(tc.tile_pool(name="loop", bufs=2))
        stats = ctx_stack.enter_context(tc.tile_pool(name="stats", bufs=1))

        def ps_tile(p, f):
            return psum.tile([p, f], F32, tag="mm", name="mm")

        # ---------------- constants ----------------
        wq_sb = [const.tile([cs, C], F32R, tag=f"wq{i}", name=f"wq{i}") for i, (c0, cs) in enumerate(CCH)]
        wo_sb = [const.tile([cs, C], F32R, tag=f"wo{i}", name=f"wo{i}") for i, (c0, cs) in enumerate(CCH)]
        b6_sb = [const.tile([cs, 6], F32, tag=f"b6{i}", name=f"b6{i}") for i, (c0, cs) in enumerate(CCH)]
        gm_sb = [const.tile([cs, G], F32, tag=f"gm{i}", name=f"gm{i}") for i, (c0, cs) in enumerate(CCH)]
        gmt_sb = [const.tile([G, cs], F32, tag=f"gmt{i}", name=f"gmt{i}") for i, (c0, cs) in enumerate(CCH)]
        ones_sb = const.tile([L, L], F32R, tag="ones77", name="ones77")
        id_sb = const.tile([128, 128], F32, tag="ident", name="ident")
        for i, (c0, cs) in enumerate(CCH):
            nc.sync.dma_start(out=wq_sb[i], in_=wqt_d.ap()[c0:c0 + cs, :])
            nc.sync.dma_start(out=wo_sb[i], in_=wot_d.ap()[c0:c0 + cs, :])
            nc.sync.dma_start(out=b6_sb[i], in_=bias6_d.ap()[c0:c0 + cs, :])
            nc.sync.dma_start(out=gm_sb[i], in_=gmap_d.ap()[c0:c0 + cs, :])
            nc.sync.dma_start(out=gmt_sb[i], in_=gmapt_d.ap()[:, c0:c0 + cs])
        nc.sync.dma_start(out=ones_sb, in_=ones77_d.ap())
        nc.sync.dma_start(out=id_sb, in_=ident_d.ap())

        # ---------------- prologue: k, v, vT ----------------
        k_sb = [kvres.tile([cs, L], F32R, tag=f"k{i}", name=f"k{i}") for i, (c0, cs) in enumerate(CCH)]
        vt_sb = [kvres.tile([L, cs], F32R, tag=f"vt{i}", name=f"vt{i}") for i, (c0, cs) in enumerate(CCH)]

        with tc.tile_pool(name="prol", bufs=1) as prol:
            kv_ps = [ps_tile(128, L) for _ in range(6)]
            for j in range(6):
                cj_in = prol.tile([L, 128], F32, tag="cj_in", name="cj_in", bufs=2)
                nc.sync.dma_start(out=cj_in, in_=ctx_d.ap()[:, j * 128:(j + 1) * 128])
                tp = ps_tile(128, L)
                nc.tensor.transpose(tp, cj_in, id_sb[0:L, 0:L])
                cj = prol.tile([128, L], F32, tag="ctxt", name="ctxt", bufs=2)
                nc.scalar.activation(cj, tp, AF.Copy)
                wkv_j = prol.tile([128, KV_COLS], F32, tag="wkv", name="wkv", bufs=1)
                nc.sync.dma_start(out=wkv_j, in_=wkvt_d.ap()[j * 128:(j + 1) * 128, :])
                for m in range(6):
                    nc.tensor.matmul(kv_ps[m], wkv_j[:, m * 128:(m + 1) * 128], cj,
                                     start=(j == 0), stop=(j == 5))
            v_sb = []
            for m in range(6):
                if m < 3:
                    c0, cs = CCH[m]
                    nc.scalar.activation(k_sb[m], kv_ps[m][0:cs, :], AF.Identity,
                                         bias=b6_sb[m][:, BK:BK + 1])
                else:
                    c0, cs = CCH[m - 3]
                    vm = prol.tile([cs, L], F32, tag=f"v{m - 3}", name=f"v{m - 3}")
                    nc.scalar.activation(vm, kv_ps[m][0:cs, :], AF.Identity,
                                         bias=b6_sb[m - 3][:, BV:BV + 1])
                    v_sb.append(vm)
            # vT chunks
            for i, (c0, cs) in enumerate(CCH):
                tp = ps_tile(L, 128)
                nc.tensor.transpose(tp[:, 0:cs], v_sb[i], id_sb[0:cs, 0:cs])
                nc.scalar.activation(vt_sb[i], tp[:, 0:cs], AF.Copy)

        # ---------------- resident x chunks + main loop ----------------
        xb = [xpool.tile([cs, N], F32, tag=f"xb{i}", name=f"xb{i}") for i, (c0, cs) in enumerate(CCH)]
        st_sb = [stats.tile([cs, NT, 6], F32, tag=f"st{i}", name=f"st{i}") for i, (c0, cs) in enumerate(CCH)]
        for it in range(NT):
            n0 = it * N_TILE
            nsl = slice(n0, n0 + N_TILE)
            for i, (c0, cs) in enumerate(CCH):
                nc.sync.dma_start(out=xb[i][:, nsl], in_=x_d.ap()[c0:c0 + cs, nsl])
            # rounded fp32r copies of x tile for the q-proj matmuls
            xq = []
            for i, (c0, cs) in enumerate(CCH):
                xq_i = loop.tile([cs, N_TILE], F32R, tag=f"xq{i}", name=f"xq{i}")
                nc.gpsimd.tensor_copy(xq_i, xb[i][:, nsl])
                xq.append(xq_i)
            # q projection: q[m] = sum_k wq[k][:, m].T @ x[k]
            q_sb = []
            for m, (m0, ms) in enumerate(CCH):
                qp = ps_tile(ms, N_TILE)
                for ki in range(3):
                    nc.tensor.matmul(qp, wq_sb[ki][:, m0:m0 + ms], xq[ki],
                                     start=(ki == 0), stop=(ki == 2))
                qm = loop.tile([ms, N_TILE], F32R, tag=f"q{m}", name=f"q{m}")
                nc.scalar.activation(qm, qp, AF.Identity, bias=b6_sb[m][:, BQ:BQ + 1])
                q_sb.append(qm)
            # scores [77, n] = sum_k k_sb[k].T @ q[k]   (scale already in Wq)
            sp = ps_tile(L, N_TILE)
            for ki in range(3):
                nc.tensor.matmul(sp, k_sb[ki], q_sb[ki], start=(ki == 0), stop=(ki == 2))
            u = loop.tile([L, N_TILE], F32R, tag="u", name="u")
            nc.scalar.activation(u, sp, AF.Exp)
            # colsum of u broadcast to 77 partitions via all-ones lhsT
            cb = ps_tile(L, N_TILE)
            nc.tensor.matmul(cb, ones_sb, u, start=True, stop=True)
            rb = loop.tile([L, N_TILE], F32, tag="rb", name="rb")
            nc.vector.reciprocal(out=rb, in_=cb)
            nc.vector.tensor_mul(u, u.bitcast(F32), rb)
            # h[m] = vT[m].T @ un ; copy to SBUF
            h_sb = []
            for m, (m0, ms) in enumerate(CCH):
                hp = ps_tile(ms, N_TILE)
                nc.tensor.matmul(hp, vt_sb[m], u, start=True, stop=True)
                hm = loop.tile([ms, N_TILE], F32R, tag=f"h{m}", name=f"h{m}")
                nc.scalar.activation(hm, hp, AF.Copy)
                h_sb.append(hm)
            # o projection + residual + stats
            for m, (m0, ms) in enumerate(CCH):
                op = ps_tile(ms, N_TILE)
                for ki in range(3):
                    nc.tensor.matmul(op, wo_sb[ki][:, m0:m0 + ms], h_sb[ki],
                                     start=(ki == 0), stop=(ki == 2))
                xmv = xb[m][:, nsl]
                nc.vector.tensor_add(xmv, op, xmv)
                nc.vector.bn_stats(out=st_sb[m][:, it, :], in_=xmv)

        # ---------------- groupnorm stats ----------------
        sm = ctx_stack.enter_context(tc.tile_pool(name="sm", bufs=1))
        st3 = []
        for i, (c0, cs) in enumerate(CCH):
            mv = sm.tile([cs, 2], F32, tag=f"mv{i}", name=f"mv{i}")
            nc.vector.bn_aggr(out=mv, in_=st_sb[i])
            s3 = sm.tile([cs, 3], F32, tag=f"s3{i}", name=f"s3{i}")
            # mean' = mean + bo ; var ; mean'^2
            nc.vector.tensor_add(s3[:, 0:1], mv[:, 0:1], b6_sb[i][:, BO:BO + 1])
            nc.vector.tensor_copy(s3[:, 1:2], mv[:, 1:2])
            nc.vector.tensor_mul(s3[:, 2:3], s3[:, 0:1], s3[:, 0:1])
            st3.append(s3)
        gp = ps_tile(G, 3)
        for i in range(3):
            nc.tensor.matmul(gp, gm_sb[i], st3[i], start=(i == 0), stop=(i == 2))
        # group stats: mu = s_mean/GS ; var = (s_var + s_mean2)/GS - mu^2
        gs = sm.tile([G, 3], F32, tag="gs", name="gs")
        nc.scalar.activation(gs, gp, AF.Copy)
        mu = sm.tile([G, 1], F32, tag="mu", name="mu")
        nc.scalar.activation(mu, gs[:, 0:1], AF.Copy, scale=1.0 / GS)
        tvar = sm.tile([G, 1], F32, tag="tvar", name="tvar")
        nc.vector.tensor_add(tvar, gs[:, 1:2], gs[:, 2:3])
        mu2 = sm.tile([G, 1], F32, tag="mu2", name="mu2")
        nc.vector.tensor_mul(mu2, mu, mu)
        var = sm.tile([G, 1], F32, tag="var", name="var")
        nc.vector.scalar_tensor_tensor(
            out=var, in0=tvar, scalar=1.0 / GS, in1=mu2,
            op0=ALU.mult, op1=ALU.subtract)
        # rstd = 1/sqrt(var + eps)
        epsb = sm.tile([G, 1], F32, tag="epsb", name="epsb")
        nc.vector.memset(epsb, EPS)
        sd = sm.tile([G, 1], F32, tag="sd", name="sd")
        nc.scalar.activation(sd, var, AF.Sqrt, bias=epsb)
        rstd = sm.tile([G, 1], F32, tag="rstd", name="rstd")
        nc.vector.reciprocal(out=rstd, in_=sd)
        mr = sm.tile([G, 2], F32, tag="mr", name="mr")
        nc.vector.tensor_copy(mr[:, 0:1], mu)
        nc.vector.tensor_copy(mr[:, 1:2], rstd)
        # broadcast back per channel: [cs, 2] = gmapT[m].T @ mr
        ab = []
        for m, (m0, ms) in enumerate(CCH):
            bp = ps_tile(ms, 2)
            nc.tensor.matmul(bp, gmt_sb[m], mr, start=True, stop=True)
            a_m = sm.tile([ms, 1], F32, tag=f"a{m}", name=f"a{m}")
            nc.vector.tensor_mul(a_m, bp[:, 1:2], b6_sb[m][:, GA:GA + 1])
            # b = beta + a*(bo - mu)
            t1 = sm.tile([ms, 1], F32, tag=f"t1{m}", name=f"t1{m}")
            nc.vector.tensor_sub(t1, b6_sb[m][:, BO:BO + 1], bp[:, 0:1])
            t2 = sm.tile([ms, 1], F32, tag=f"t2{m}", name=f"t2{m}")
            nc.vector.tensor_mul(t2, t1, a_m)
            b_m = sm.tile([ms, 1], F32, tag=f"b{m}", name=f"b{m}")
            nc.vector.tensor_add(b_m, b6_sb[m][:, BE:BE + 1], t2)
            ab.append((a_m, b_m))

        # ---------------- pass 2: swish + store ----------------
        for it in range(NT):
            n0 = it * N_TILE
            nsl = slice(n0, n0 + N_TILE)
            for m, (m0, ms) in enumerate(CCH):
                a_m, b_m = ab[m]
                xmv = xb[m][:, nsl]
                if USE_SILU:
                    nc.scalar.activation(xmv, xmv, AF.Silu, bias=b_m, scale=a_m)
                else:
                    # sim fallback: xn*sigmoid(xn) with xn = a*x + b, via
                    #   sig = sigmoid(a*x+b); t = (x*a)*sig; out = (sig*b) + t
                    sgm = loop.tile([ms, N_TILE], F32, tag="sg", name="sg", bufs=1)
                    nc.scalar.activation(sgm, xmv, AF.Sigmoid, bias=b_m, scale=a_m)
                    nc.vector.scalar_tensor_tensor(
                        out=xmv, in0=xmv, scalar=a_m, in1=sgm,
                        op0=ALU.mult, op1=ALU.mult)
                    nc.vector.scalar_tensor_tensor(
                        out=xmv, in0=sgm, scalar=b_m, in1=xmv,
                        op0=ALU.mult, op1=ALU.add)
                nc.sync.dma_start(out=out_d.ap()[m0:m0 + ms, nsl], in_=xb[m][:, nsl])


_NC_CACHE = None


def _get_nc():
    global _NC_CACHE
    if _NC_CACHE is None:
        _NC_CACHE = _build()
    return _NC_CACHE


def _host_consts(Wq, bq, Wk, bk, Wv, bv, Wo, bo, gamma, beta):
    s = float(C) ** -0.5
    wqt = np.ascontiguousarray((Wq * s).T.astype(np.float32))
    wot = np.ascontiguousarray(Wo.T.astype(np.float32))
    wkvt = np.zeros((CTX, KV_COLS), np.float32)
    wkt = Wk.T.astype(np.float32)   # [CTX, C]
    wvt = Wv.T.astype(np.float32)
    wkvt[:, 0:128] = wkt[:, 0:128]
    wkvt[:, 128:256] = wkt[:, 128:256]
    wkvt[:, 256:320] = wkt[:, 256:320]
    wkvt[:, 384:512] = wvt[:, 0:128]
    wkvt[:, 512:640] = wvt[:, 128:256]
    wkvt[:, 640:704] = wvt[:, 256:320]
    bias6 = np.stack([bq * s, bk, bv, bo, gamma, beta], axis=1).astype(np.float32)
    bias6 = np.ascontiguousarray(bias6)
    gmap = np.zeros((C, G), np.float32)
    gmap[np.arange(C), np.arange(C) // GS] = 1.0
    gmapt = np.ascontiguousarray(gmap.T)
    ones77 = np.ones((L, L), np.float32)
    ident = np.eye(128, dtype=np.float32)
    return dict(wqt=wqt, wot=wot, wkvt=wkvt, bias6=bias6, gmap=gmap,
                gmapt=gmapt, ones77=ones77, ident=ident)


def kernel(x, context, Wq, bq, Wk, bk, Wv, bv, Wo, bo, gamma, beta,
           _return_results=False, _trace=False):
    x = np.asarray(x, np.float32)
    context = np.asarray(context, np.float32)
    consts = _host_consts(np.asarray(Wq, np.float32), np.asarray(bq, np.float32),
                          np.asarray(Wk, np.float32), np.asarray(bk, np.float32),
                          np.asarray(Wv, np.float32), np.asarray(bv, np.float32),
                          np.asarray(Wo, np.float32), np.asarray(bo, np.float32),
                          np.asarray(gamma, np.float32), np.asarray(beta, np.float32))
    nc = _get_nc()
    in_maps = []
    for b in range(B):
        m = dict(consts)
        m["x"] = np.ascontiguousarray(x[b].reshape(C, N))
        m["ctx"] = np.ascontiguousarray(context[b])
        in_maps.append(m)
    res = run_bass_kernel_spmd(nc, in_maps, core_ids=list(range(B)), trace=_trace)
    out = np.stack([res.results[b]["out"].reshape(C, D, D, D) for b in range(B)])
    if _return_results:
        return out, res
    return out



# revision 33
# speedup vs baseline: 3.0258x; 3.0258x over previous
"""Trainium2 Bass kernel for CrossAttention + GroupNorm + Swish (nn_CrossAttention).

Reference computation (per batch element b, xf = x[b] reshaped [C, N]):
    q  = Wq @ xf + bq                       [C, N]
    k  = Wk @ ctx^T + bk                    [C, L]
    v  = Wv @ ctx^T + bv                    [C, L]
    qk = (q^T k) * C^-0.5                   [N, L]
    w  = softmax(qk, axis=-1)
    h  = v @ w^T                            [C, N]
    o  = Wo @ h + bo
    xr = o + xf
    out = swish(groupnorm(xr; 32 groups over (C/32, N)) * gamma + beta)

Sharding: data-parallel over batch B=8 across the 8 NeuronCores (no collectives).

Key algebraic restructuring (L=77 << C=320 makes attention low-rank):
    scores^T = k'^T xf + blk      with k'  = (Wq*s)^T k   [C, L]   (one-time)
                                       blk = k^T (bq*s)   [L, 1]   (one-time,
                                       applied as per-partition bias in Exp)
    o        = v2t^T w            with v2t = (Wo v)^T     [L, C]   (one-time)
so the per-tile work is only: 3 score matmuls, Exp, ones-matmul colsum,
reciprocal, mul, 3 o-matmuls, residual add, bn_stats.  The q/o projections
(18 matmuls + 6 ACT ops per tile in the direct form) disappear.

Device algorithm (per core):
  - x chunks resident in SBUF ([128|128|64] x 13824 fp32), loaded in 9 large
    DMAs, updated in place with xr, stored in large DMAs after pass 2.
  - n-tiles of 512 (27 tiles); all matmuls fp32r (full PE rate, moving dim 512).
  - softmax without max-subtraction (scores tiny; scale folded into k').
  - colsum of exp-scores broadcast across partitions via one all-ones matmul.
  - normalization mul runs on GpSimd (Pool) to unload DVE.
  - GroupNorm stats via bn_stats/bn_aggr per channel + group-membership
    matmuls (gmap [C,32], gmapT [32,C]); bias bo folded analytically.
  - pass 2: out = Silu(a_c * xr + b_c) as one ACT op per [cs, 1728] slice.
"""
import sys

sys.path.insert(0, "/opt/trn_rl_repo")

import numpy as np

import concourse.tile as tile
from concourse import bacc, mybir
from concourse.bass_utils import run_bass_kernel_spmd

F32 = mybir.dt.float32
F32R = mybir.dt.float32r
AF = mybir.ActivationFunctionType
ALU = mybir.AluOpType

# Problem shapes (hardcoded; harness contract)
B, C, D, L, CTX = 8, 320, 24, 77, 768
N = D * D * D            # 13824 spatial positions
G = 32                   # groupnorm groups
GS = C // G              # 10 channels per group
EPS = 1e-5
N_TILE = 512
NT = N // N_TILE         # 27
CCH = [(0, 128), (128, 128), (256, 64)]   # channel chunks (start, size)
KV_COLS = 768            # padded concat [k0,k1,k2+pad, v0,v1,v2+pad]
X_SLICE = 2304           # x load granularity (18 DMAs)
O_SLICE = 1152           # pass-2 silu/store granularity (36 ACT ops / DMAs)

# Silu isn't implemented in CoreSim; flip for simulation runs.
USE_SILU = True

# bias6 columns
BQ, BK, BV, BO, GA, BE = range(6)


def _build():
    nc = bacc.Bacc(trn_type="TRN2", target_bir_lowering=False, debug=False)

    x_d = nc.dram_tensor("x", [C, N], F32R, kind="ExternalInput")
    ctx_d = nc.dram_tensor("ctx", [L, CTX], F32, kind="ExternalInput")
    wqo_d = nc.dram_tensor("wqo", [C, 2 * C], F32, kind="ExternalInput")
    wkvt_d = nc.dram_tensor("wkvt", [CTX, KV_COLS], F32, kind="ExternalInput")
    # one packed f32 const tensor: [ident(128) | cpk0|cpk1|cpk2 (39 each) | bqr0..2]
    cpack_d = nc.dram_tensor("cpack", [128, 128 + 3 * (7 + G) + 3], F32,
                             kind="ExternalInput")
    gmapt_d = nc.dram_tensor("gmapt", [G, C], F32, kind="ExternalInput")
    ones77_d = nc.dram_tensor("ones77", [L, L], F32R, kind="ExternalInput")
    identr_d = nc.dram_tensor("identr", [128, 128], F32R, kind="ExternalInput")
    out_d = nc.dram_tensor("out", [C, N], mybir.dt.bfloat16, kind="ExternalOutput")

    with tile.TileContext(nc) as tc:
        _emit(nc, tc, x_d, ctx_d, wqo_d, wkvt_d, cpk_d, bqr_d,
              gmapt_d, ones77_d, ident_d, identr_d, out_d)
    nc.compile()
    return nc


def _emit(nc, tc, x_d, ctx_d, wqo_d, wkvt_d, cpack_d,
          gmapt_d, ones77_d, ident_d, identr_d, out_d):
    from contextlib import ExitStack

    with ExitStack() as ctx_stack:
        const = ctx_stack.enter_context(tc.tile_pool(name="const", bufs=1))
        xpool = ctx_stack.enter_context(tc.tile_pool(name="xbuf", bufs=1))
        kvres = ctx_stack.enter_context(tc.tile_pool(name="kvres", bufs=1))
        psum = ctx_stack.enter_context(tc.tile_pool(name="psum", bufs=8, space="PSUM"))
        loop = ctx_stack.enter_context(tc.tile_pool(name="loop", bufs=2))
        stats = ctx_stack.enter_context(tc.tile_pool(name="stats", bufs=1))

        def ps_tile(p, f):
            return psum.tile([p, f], F32, tag="mm", name="mm")

        # ---------------- constants ----------------
        # DMA issue order is tuned for ramp time: ident + packed consts first
        # (prologue-critical), ctx/wkv/wqn/wot inside the prologue, then
        # ones77, then the 18 big x slices, then gmapt (needed only at stats).
        NCP = 7 + G
        cpack_sb = const.tile([128, 128 + 3 * NCP + 3], F32, tag="cpack", name="cpack")
        gmt_sb = const.tile([G, C], F32, tag="gmt", name="gmt")
        ones_sb = const.tile([L, L], F32R, tag="ones77", name="ones77")
        id_sb = cpack_sb[:, 0:128]
        cpk_sb = [cpack_sb[0:cs, 128 + i * NCP:128 + (i + 1) * NCP]
                  for i, (c0, cs) in enumerate(CCH)]
        b6_sb = cpk_sb                       # cols 0..5 = bq*s|bk|bv|bo|gamma|beta
        gm_sb = [t[:, 7:7 + G] for t in cpk_sb]
        bqr_sb = [cpack_sb[0:cs, 128 + 3 * NCP + i:128 + 3 * NCP + i + 1]
                  for i, (c0, cs) in enumerate(CCH)]
        idr_sb = const.tile([128, 128], F32R, tag="identr", name="identr")
        nc.sync.dma_start(out=cpack_sb, in_=cpack_d.ap())
        nc.sync.dma_start(out=idr_sb, in_=identr_d.ap())

        # x resident chunks (loaded below, after the prologue's DMAs are queued)
        xb = [xpool.tile([cs, N], F32R, tag=f"xb{i}", name=f"xb{i}") for i, (c0, cs) in enumerate(CCH)]

        # ---------------- prologue: k, v -> k' (kq), v2t, blk ----------------
        kq_sb = [kvres.tile([cs, L], F32R, tag=f"kq{i}", name=f"kq{i}") for i, (c0, cs) in enumerate(CCH)]
        v2t_sb = kvres.tile([L, C], F32R, tag="v2t", name="v2t")
        blk_sb = kvres.tile([L, 1], F32, tag="blk", name="blk")

        with tc.tile_pool(name="prolA", bufs=1) as prolA:
            # k/v and the fused-projection weights span both prologue phases
            k_sb = [prolA.tile([cs, L], F32, tag=f"k{i}", name=f"k{i}")
                    for i, (c0, cs) in enumerate(CCH)]
            v_sb = [prolA.tile([cs, L], F32, tag=f"v{i}", name=f"v{i}")
                    for i, (c0, cs) in enumerate(CCH)]
            wqo_sb = [prolA.tile([cs, 2 * C], F32, tag=f"wqo{i}", name=f"wqo{i}")
                      for i, (c0, cs) in enumerate(CCH)]
            wqn_sb = [t[:, 0:C] for t in wqo_sb]
            wot_sb = [t[:, C:2 * C] for t in wqo_sb]

            with tc.tile_pool(name="prolB", bufs=1) as prolB:
                kv_ps = [ps_tile(128, L) for _ in range(6)]
                for j in range(6):
                    cj_in = prolB.tile([L, 128], F32, tag="cj_in", name="cj_in", bufs=2)
                    nc.sync.dma_start(out=cj_in, in_=ctx_d.ap()[:, j * 128:(j + 1) * 128])
                    tp = ps_tile(128, L)
                    nc.tensor.transpose(tp, cj_in, id_sb[0:L, 0:L])
                    cj = prolB.tile([128, L], F32, tag="ctxt", name="ctxt", bufs=2)
                    nc.scalar.activation(cj, tp, AF.Copy)
                    wkv_j = prolB.tile([128, KV_COLS], F32, tag="wkv", name="wkv", bufs=2)
                    nc.sync.dma_start(out=wkv_j, in_=wkvt_d.ap()[j * 128:(j + 1) * 128, :])
                    for m in range(6):
                        nc.tensor.matmul(kv_ps[m], wkv_j[:, m * 128:(m + 1) * 128], cj,
                                         start=(j == 0), stop=(j == 5))
                for i, (c0, cs) in enumerate(CCH):
                    nc.sync.dma_start(out=wqo_sb[i], in_=wqo_d.ap()[c0:c0 + cs, :])
                for m in range(6):
                    if m < 3:
                        c0, cs = CCH[m]
                        nc.scalar.activation(k_sb[m], kv_ps[m][0:cs, :], AF.Identity,
                                             bias=b6_sb[m][:, BK:BK + 1])
                    else:
                        c0, cs = CCH[m - 3]
                        nc.scalar.activation(v_sb[m - 3], kv_ps[m][0:cs, :], AF.Identity,
                                             bias=b6_sb[m - 3][:, BV:BV + 1])

            # kq[m] = sum_o (Wq*s)[o, m-chunk]^T k[o, :]   -> [ms, 77]
            for m, (m0, ms) in enumerate(CCH):
                kp = ps_tile(ms, L)
                for ki in range(3):
                    nc.tensor.matmul(kp, wqn_sb[ki][:, m0:m0 + ms], k_sb[ki],
                                     start=(ki == 0), stop=(ki == 2))
                nc.scalar.activation(kq_sb[m], kp, AF.Copy)
            # v2t = (Wo v)^T = v^T Wo^T  -> [77, 320]
            vp = ps_tile(L, C)
            for ki in range(3):
                nc.tensor.matmul(vp, v_sb[ki], wot_sb[ki],
                                 start=(ki == 0), stop=(ki == 2))
            nc.scalar.activation(v2t_sb, vp, AF.Copy)
            # blk = k^T (bq*s)  -> [77, 1]
            bp = ps_tile(L, 1)
            for ki in range(3):
                nc.tensor.matmul(bp, k_sb[ki], bqr_sb[ki],
                                 start=(ki == 0), stop=(ki == 2))
            nc.scalar.activation(blk_sb, bp, AF.Copy)

        # ones77 next (needed by the first colsum), then the x slices, then
        # gmapt (only needed at the stats phase).
        nc.sync.dma_start(out=ones_sb, in_=ones77_d.ap())
        for s0 in range(0, N, X_SLICE):
            ssl = slice(s0, s0 + X_SLICE)
            for i, (c0, cs) in enumerate(CCH):
                nc.sync.dma_start(out=xb[i][:, ssl], in_=x_d.ap()[c0:c0 + cs, ssl])
        nc.sync.dma_start(out=gmt_sb, in_=gmapt_d.ap())

        # ---------------- main loop (software-pipelined, 2-tile skew) -------
        # Per tile t: FRONT = scores+exp+colsum (PE/ACT), MID = recip+mul
        # (DVE/Pool) one tile behind, TAIL = o-matmul+residual+copy+stats
        # (PE/ACT/DVE) two tiles behind.  The stage skew keeps every engine's
        # in-order queue fed with ready instructions.
        st_sb = [stats.tile([cs, NT, 6], F32, tag=f"st{i}", name=f"st{i}") for i, (c0, cs) in enumerate(CCH)]
        u_t, cb_t, rb_t = {}, {}, {}

        def front(it):
            nsl = slice(it * N_TILE, (it + 1) * N_TILE)
            # scores [77, n] = sum_k kq[k].T @ x[k]  (q-proj folded into kq)
            sp = ps_tile(L, N_TILE)
            for ki in range(3):
                nc.tensor.matmul(sp, kq_sb[ki], xb[ki][:, nsl],
                                 start=(ki == 0), stop=(ki == 2))
            u = loop.tile([L, N_TILE], F32R, tag="u", name="u", bufs=3)
            nc.scalar.activation(u, sp, AF.Exp, bias=blk_sb)
            u_t[it] = u

        def colsum(it):
            # colsum of u broadcast to 77 partitions via all-ones lhsT
            cb = ps_tile(L, N_TILE)
            nc.tensor.matmul(cb, ones_sb, u_t[it], start=True, stop=True)
            cb_t[it] = cb

        def mid(it):
            rb = loop.tile([L, N_TILE], F32, tag="rb", name="rb", bufs=2)
            nc.vector.reciprocal(out=rb, in_=cb_t.pop(it))
            nc.gpsimd.tensor_mul(u_t[it], u_t[it].bitcast(F32), rb)

        def tail(it):
            nsl = slice(it * N_TILE, (it + 1) * N_TILE)
            u = u_t.pop(it)
            # xr[m] = v2t[:, m-chunk].T @ u + x[m]  (residual accumulated on
            # PE via identity matmul); ACT copies PSUM->SBUF in place, DVE
            # only does bn_stats.
            ops = []
            for m, (m0, ms) in enumerate(CCH):
                op = ps_tile(ms, N_TILE)
                nc.tensor.matmul(op, v2t_sb[:, m0:m0 + ms], u, start=True, stop=False)
                nc.tensor.matmul(op, idr_sb[0:ms, 0:ms], xb[m][:, nsl],
                                 start=False, stop=True)
                ops.append(op)
            for m, (m0, ms) in enumerate(CCH):
                nc.scalar.activation(xb[m][:, nsl], ops[m], AF.Copy)
            for m, (m0, ms) in enumerate(CCH):
                nc.vector.bn_stats(out=st_sb[m][:, it, :],
                                   in_=xb[m][:, nsl].bitcast(F32))

        for it in range(NT + 2):
            if it - 2 >= 0:
                tail(it - 2)
            if it < NT:
                front(it)
            if 0 <= it - 1 < NT:
                mid(it - 1)
            if it < NT:
                colsum(it)

        # ---------------- groupnorm stats ----------------
        sm = ctx_stack.enter_context(tc.tile_pool(name="sm", bufs=1))
        st3 = []
        for i, (c0, cs) in enumerate(CCH):
            mv = sm.tile([cs, 2], F32, tag=f"mv{i}", name=f"mv{i}")
            nc.vector.bn_aggr(out=mv, in_=st_sb[i])
            s3 = sm.tile([cs, 3], F32, tag=f"s3{i}", name=f"s3{i}")
            # mean' = mean + bo ; var ; mean'^2
            nc.vector.tensor_add(s3[:, 0:1], mv[:, 0:1], b6_sb[i][:, BO:BO + 1])
            nc.vector.tensor_copy(s3[:, 1:2], mv[:, 1:2])
            nc.vector.tensor_mul(s3[:, 2:3], s3[:, 0:1], s3[:, 0:1])
            st3.append(s3)
        gp = ps_tile(G, 3)
        for i in range(3):
            nc.tensor.matmul(gp, gm_sb[i], st3[i], start=(i == 0), stop=(i == 2))
        # group stats: mu = s_mean/GS ; var = (s_var + s_mean2)/GS - mu^2
        gs = sm.tile([G, 3], F32, tag="gs", name="gs")
        nc.scalar.activation(gs, gp, AF.Copy)
        mu = sm.tile([G, 1], F32, tag="mu", name="mu")
        nc.scalar.activation(mu, gs[:, 0:1], AF.Copy, scale=1.0 / GS)
        tvar = sm.tile([G, 1], F32, tag="tvar", name="tvar")
        nc.vector.tensor_add(tvar, gs[:, 1:2], gs[:, 2:3])
        mu2 = sm.tile([G, 1], F32, tag="mu2", name="mu2")
        nc.vector.tensor_mul(mu2, mu, mu)
        var = sm.tile([G, 1], F32, tag="var", name="var")
        nc.vector.scalar_tensor_tensor(
            out=var, in0=tvar, scalar=1.0 / GS, in1=mu2,
            op0=ALU.mult, op1=ALU.subtract)
        # rstd = 1/sqrt(var + eps)
        epsb = sm.tile([G, 1], F32, tag="epsb", name="epsb")
        nc.vector.memset(epsb, EPS)
        sd = sm.tile([G, 1], F32, tag="sd", name="sd")
        nc.scalar.activation(sd, var, AF.Sqrt, bias=epsb)
        rstd = sm.tile([G, 1], F32, tag="rstd", name="rstd")
        nc.vector.reciprocal(out=rstd, in_=sd)
        mr = sm.tile([G, 2], F32, tag="mr", name="mr")
        nc.vector.tensor_copy(mr[:, 0:1], mu)
        nc.vector.tensor_copy(mr[:, 1:2], rstd)
        # broadcast back per channel: [cs, 2] = gmapT[m].T @ mr
        ab = []
        for m, (m0, ms) in enumerate(CCH):
            bp = ps_tile(ms, 2)
            nc.tensor.matmul(bp, gmt_sb[:, m0:m0 + ms], mr, start=True, stop=True)
            a_m = sm.tile([ms, 1], F32, tag=f"a{m}", name=f"a{m}")
            nc.vector.tensor_mul(a_m, bp[:, 1:2], b6_sb[m][:, GA:GA + 1])
            # b = beta + a*(bo - mu)
            t1 = sm.tile([ms, 1], F32, tag=f"t1{m}", name=f"t1{m}")
            nc.vector.tensor_sub(t1, b6_sb[m][:, BO:BO + 1], bp[:, 0:1])
            t2 = sm.tile([ms, 1], F32, tag=f"t2{m}", name=f"t2{m}")
            nc.vector.tensor_mul(t2, t1, a_m)
            b_m = sm.tile([ms, 1], F32, tag=f"b{m}", name=f"b{m}")
            nc.vector.tensor_add(b_m, b6_sb[m][:, BE:BE + 1], t2)
            ab.append((a_m, b_m))

        # ---------------- pass 2: swish + store (bf16 out halves DMA) -------
        BF16 = mybir.dt.bfloat16
        for s0 in range(0, N, O_SLICE):
            ssl = slice(s0, s0 + O_SLICE)
            for m, (m0, ms) in enumerate(CCH):
                a_m, b_m = ab[m]
                xmv = xb[m][:, ssl].bitcast(F32)
                ob = loop.tile([ms, O_SLICE], BF16, tag="ob", name="ob", bufs=3)
                if USE_SILU:
                    nc.scalar.activation(ob, xmv, AF.Silu, bias=b_m, scale=a_m)
                else:
                    # sim fallback: xn*sigmoid(xn) with xn = a*x + b, via
                    #   sig = sigmoid(a*x+b); t = (x*a)*sig; out = (sig*b) + t
                    sgm = loop.tile([ms, O_SLICE], F32, tag="sg", name="sg", bufs=1)
                    nc.scalar.activation(sgm, xmv, AF.Sigmoid, bias=b_m, scale=a_m)
                    nc.vector.scalar_tensor_tensor(
                        out=xmv, in0=xmv, scalar=a_m, in1=sgm,
                        op0=ALU.mult, op1=ALU.mult)
                    nc.vector.scalar_tensor_tensor(
                        out=xmv, in0=sgm, scalar=b_m, in1=xmv,
                        op0=ALU.mult, op1=ALU.add)
                    nc.vector.tensor_copy(ob, xmv)
                nc.sync.dma_start(out=out_d.ap()[m0:m0 + ms, ssl], in_=ob)


_NC_CACHE = None


def _get_nc():
    global _NC_CACHE
    if _NC_CACHE is None:
        _NC_CACHE = _build()
    return _NC_CACHE


def _host_consts(Wq, bq, Wk, bk, Wv, bv, Wo, bo, gamma, beta):
    s = float(C) ** -0.5
    wqo = np.concatenate([(Wq * s).astype(np.float32),
                          Wo.T.astype(np.float32)], axis=1)
    wqo = np.ascontiguousarray(wqo)
    wkvt = np.zeros((CTX, KV_COLS), np.float32)
    wkt = Wk.T.astype(np.float32)   # [CTX, C]
    wvt = Wv.T.astype(np.float32)
    wkvt[:, 0:128] = wkt[:, 0:128]
    wkvt[:, 128:256] = wkt[:, 128:256]
    wkvt[:, 256:320] = wkt[:, 256:320]
    wkvt[:, 384:512] = wvt[:, 0:128]
    wkvt[:, 512:640] = wvt[:, 128:256]
    wkvt[:, 640:704] = wvt[:, 256:320]
    bias6 = np.stack([bq * s, bk, bv, bo, gamma, beta], axis=1).astype(np.float32)
    gmap = np.zeros((C, G), np.float32)
    gmap[np.arange(C), np.arange(C) // GS] = 1.0
    # per-channel consts [bias6 | bq*s | gmap], packed per chunk into cpack
    cpk = np.concatenate(
        [bias6, (bq * s).astype(np.float32).reshape(C, 1), gmap], axis=1)
    NCP = 7 + G
    cpack = np.zeros((128, 128 + 3 * NCP + 3), np.float32)
    cpack[:, 0:128] = np.eye(128, dtype=np.float32)
    for i, (c0, cs) in enumerate(CCH):
        cpack[0:cs, 128 + i * NCP:128 + (i + 1) * NCP] = cpk[c0:c0 + cs, :]
        cpack[0:cs, 128 + 3 * NCP + i] = (bq[c0:c0 + cs] * s).astype(np.float32)
    gmapt = np.ascontiguousarray(gmap.T)
    ones77 = np.ones((L, L), np.float32)
    ident = np.eye(128, dtype=np.float32)
    return dict(wqo=wqo, wkvt=wkvt, cpack=np.ascontiguousarray(cpack),
                gmapt=gmapt, ones77=ones77, identr=ident)


def kernel(x, context, Wq, bq, Wk, bk, Wv, bv, Wo, bo, gamma, beta,
           _return_results=False, _trace=False):
    x = np.asarray(x, np.float32)
    context = np.asarray(context, np.float32)
    consts = _host_consts(np.asarray(Wq, np.float32), np.asarray(bq, np.float32),
                          np.asarray(Wk, np.float32), np.asarray(bk, np.float32),
                          np.asarray(Wv, np.float32), np.asarray(bv, np.float32),
                          np.asarray(Wo, np.float32), np.asarray(bo, np.float32),
                          np.asarray(gamma, np.float32), np.asarray(beta, np.float32))
    nc = _get_nc()
    in_maps = []
    for b in range(B):
        m = dict(consts)
        m["x"] = np.ascontiguousarray(x[b].reshape(C, N))
        m["ctx"] = np.ascontiguousarray(context[b])
        in_maps.append(m)
    res = run_bass_kernel_spmd(nc, in_maps, core_ids=list(range(B)), trace=_trace)
    out = np.stack([np.asarray(res.results[b]["out"], dtype=np.float32)
                    .reshape(C, D, D, D) for b in range(B)])
    if _return_results:
        return out, res
    return out


# revision 37
# speedup vs baseline: 8.4645x; 2.7974x over previous
"""Trainium2 Bass kernel for CrossAttention + GroupNorm + Swish (nn_CrossAttention).

Reference computation (per batch element b, xf = x[b] reshaped [C, N]):
    q  = Wq @ xf + bq                       [C, N]
    k  = Wk @ ctx^T + bk                    [C, L]
    v  = Wv @ ctx^T + bv                    [C, L]
    qk = (q^T k) * C^-0.5                   [N, L]
    w  = softmax(qk, axis=-1)
    h  = v @ w^T                            [C, N]
    o  = Wo @ h + bo
    xr = o + xf
    out = swish(groupnorm(xr; 32 groups over (C/32, N)) * gamma + beta)

Sharding: data-parallel over batch B=8 across the 8 NeuronCores (no collectives).

Key algebraic restructuring (L=77 << C=320 makes attention low-rank):
    scores^T = k'^T xf + blk      with k'  = (Wq*s)^T k   [C, L]   (one-time)
                                       blk = k^T (bq*s)   [L, 1]   (one-time,
                                       applied as per-partition bias in Exp)
    o        = v2t^T w            with v2t = (Wo v)^T     [L, C]   (one-time)
so the per-tile work is only: 3 score matmuls, Exp, ones-matmul colsum,
reciprocal, mul, 3 o-matmuls, residual add, bn_stats.  The q/o projections
(18 matmuls + 6 ACT ops per tile in the direct form) disappear.

Device algorithm (per core):
  - x chunks resident in SBUF ([128|128|64] x 13824 fp32r), loaded in 21
    large DMAs (two small leading slices for a fast ramp), updated in place
    with xr, stored as bf16 after pass 2 (tolerance 2e-2 >> bf16 rounding).
  - n-tiles of 512 (27 tiles); main-loop matmuls fp32r (full PE rate at
    moving dim 512); tiny prologue matmuls plain fp32 (fp32r has ISA
    restrictions at odd/small moving dims).
  - the residual add rides the PE: the o-matmul PSUM group accumulates an
    identity matmul of x, so ACT's PSUM->SBUF copy IS the residual write;
    DVE only does softmax reciprocal + bn_stats, Pool does the softmax mul.
  - main loop is software-pipelined 4 stages deep (front: scores+exp,
    mid(-1): recip+mul, tail(-2): o-matmul+copy, bn(-3)) so every engine's
    in-order queue only sees ready instructions.
  - softmax without max-subtraction (scores tiny; scale folded into k').
  - colsum of exp-scores broadcast across partitions via one all-ones matmul.
  - GroupNorm stats via bn_stats/bn_aggr per channel + group-membership
    matmuls (gmap [C,32], gmapT [32,C]); bias bo folded analytically;
    1/sqrt(var+eps) via DVE bit-trick + Newton (avoids an ACT table swap).
  - pass 2: out = Silu(a_c * xr + b_c) as one ACT op per [cs, 1728] slice,
    written to bf16 staging tiles and DMA'd out (half the store traffic).
  - consts are packed ([ident|bias6|bq*s|gmap|rsqrt magics] in one tensor,
    Wq*s and Wo^T concatenated) to minimize serialized HWDGE descriptor time
    during the ramp.
"""
import sys

sys.path.insert(0, "/opt/trn_rl_repo")

import numpy as np

import concourse.tile as tile
from concourse import bacc, mybir
from concourse.bass_utils import run_bass_kernel_spmd

F32 = mybir.dt.float32
F32R = mybir.dt.float32r
AF = mybir.ActivationFunctionType
ALU = mybir.AluOpType

# Problem shapes (hardcoded; harness contract)
B, C, D, L, CTX = 8, 320, 24, 77, 768
N = D * D * D            # 13824 spatial positions
G = 32                   # groupnorm groups
GS = C // G              # 10 channels per group
EPS = 1e-5
N_TILE = 512
NT = N // N_TILE         # 27
CCH = [(0, 128), (128, 128), (256, 64)]   # channel chunks (start, size)
KV_COLS = 768            # padded concat [k0,k1,k2+pad, v0,v1,v2+pad]
X_WIDTHS = [1152, 1152] + [2304] * 5   # x load slices (21 DMAs, fast start)
O_SLICE = 1728           # pass-2 silu/store granularity (24 ACT ops / DMAs)

# Silu isn't implemented in CoreSim; flip for simulation runs.
USE_SILU = True

# bias6 columns
BQ, BK, BV, BO, GA, BE = range(6)


def _build(reps=1):
    nc = bacc.Bacc(trn_type="TRN2", target_bir_lowering=False, debug=False)

    x_d = nc.dram_tensor("x", [C, N], F32R, kind="ExternalInput")
    ctx_d = nc.dram_tensor("ctx", [L, CTX], F32, kind="ExternalInput")
    wqo_d = nc.dram_tensor("wqo", [C, 2 * C], F32, kind="ExternalInput")
    wkvt_d = nc.dram_tensor("wkvt", [CTX, KV_COLS], F32, kind="ExternalInput")
    # one packed f32 const tensor: [ident(128) | cpk0|cpk1|cpk2 (39 each) | bqr0..2]
    cpack_d = nc.dram_tensor("cpack", [128, 128 + 3 * (7 + G) + 5], F32,
                             kind="ExternalInput")
    gmapt_d = nc.dram_tensor("gmapt", [G, C], F32, kind="ExternalInput")
    ones77_d = nc.dram_tensor("ones77", [L, L], F32R, kind="ExternalInput")
    identr_d = nc.dram_tensor("identr", [128, 128], F32R, kind="ExternalInput")
    out_d = nc.dram_tensor("out", [C, N], mybir.dt.bfloat16, kind="ExternalOutput")

    with tile.TileContext(nc) as tc:
        _emit(nc, tc, x_d, ctx_d, wqo_d, wkvt_d, cpack_d,
              gmapt_d, ones77_d, identr_d, out_d, reps)
    nc.compile()
    return nc


def _emit(nc, tc, x_d, ctx_d, wqo_d, wkvt_d, cpack_d,
          gmapt_d, ones77_d, identr_d, out_d, reps=1):
    from contextlib import ExitStack

    with ExitStack() as ctx_stack:
        const = ctx_stack.enter_context(tc.tile_pool(name="const", bufs=1))
        xpool = ctx_stack.enter_context(tc.tile_pool(name="xbuf", bufs=1))
        kvres = ctx_stack.enter_context(tc.tile_pool(name="kvres", bufs=1))
        loop = ctx_stack.enter_context(tc.tile_pool(name="loop", bufs=2))
        stats = ctx_stack.enter_context(tc.tile_pool(name="stats", bufs=1))

        psum = ctx_stack.enter_context(tc.tile_pool(name="psum", bufs=8, space="PSUM"))

        def ps_tile(p, f):
            return psum.tile([p, f], F32, tag="mm", name="mm")

        # ---------------- constants ----------------
        # DMA issue order is tuned for ramp time: ident + packed consts first
        # (prologue-critical), ctx/wkv/wqn/wot inside the prologue, then
        # ones77, then the 18 big x slices, then gmapt (needed only at stats).
        NCP = 7 + G
        cpack_sb = const.tile([128, 128 + 3 * NCP + 5], F32, tag="cpack", name="cpack")
        gmt_sb = const.tile([G, C], F32, tag="gmt", name="gmt")
        ones_sb = const.tile([L, L], F32R, tag="ones77", name="ones77")
        id_sb = cpack_sb[:, 0:128]
        cpk_sb = [cpack_sb[0:cs, 128 + i * NCP:128 + (i + 1) * NCP]
                  for i, (c0, cs) in enumerate(CCH)]
        b6_sb = cpk_sb                       # cols 0..5 = bq*s|bk|bv|bo|gamma|beta
        gm_sb = [t[:, 7:7 + G] for t in cpk_sb]
        bqr_sb = [cpack_sb[0:cs, 128 + 3 * NCP + i:128 + 3 * NCP + i + 1]
                  for i, (c0, cs) in enumerate(CCH)]
        cmagic_sb = cpack_sb[0:G, 128 + 3 * NCP + 3:128 + 3 * NCP + 4].bitcast(mybir.dt.int32)
        c15_sb = cpack_sb[0:G, 128 + 3 * NCP + 4:128 + 3 * NCP + 5]
        idr_sb = const.tile([128, 128], F32R, tag="identr", name="identr")
        nc.sync.dma_start(out=cpack_sb, in_=cpack_d.ap())
        nc.sync.dma_start(out=idr_sb, in_=identr_d.ap())

        # x resident chunks (loaded below, after the prologue's DMAs are queued)
        xb = [xpool.tile([cs, N], F32R, tag=f"xb{i}", name=f"xb{i}")
              for i, (c0, cs) in enumerate(CCH)]

        def xv(m, sl):
            return xb[m][:, sl]

        # ---------------- prologue: k, v -> k' (kq), v2t, blk ----------------
        kq_sb = [kvres.tile([cs, L], F32R, tag=f"kq{i}", name=f"kq{i}") for i, (c0, cs) in enumerate(CCH)]
        v2t_sb = kvres.tile([L, C], F32R, tag="v2t", name="v2t")
        blk_sb = kvres.tile([L, 1], F32, tag="blk", name="blk")

        with tc.tile_pool(name="prolA", bufs=1) as prolA:
            # k/v and the fused-projection weights span both prologue phases
            k_sb = [prolA.tile([cs, L], F32, tag=f"k{i}", name=f"k{i}")
                    for i, (c0, cs) in enumerate(CCH)]
            v_sb = [prolA.tile([cs, L], F32, tag=f"v{i}", name=f"v{i}")
                    for i, (c0, cs) in enumerate(CCH)]
            wqo_sb = [prolA.tile([cs, 2 * C], F32, tag=f"wqo{i}", name=f"wqo{i}")
                      for i, (c0, cs) in enumerate(CCH)]
            wqn_sb = [t[:, 0:C] for t in wqo_sb]
            wot_sb = [t[:, C:2 * C] for t in wqo_sb]

            with tc.tile_pool(name="prolB", bufs=1) as prolB:
                kv_ps = [ps_tile(128, L) for _ in range(6)]
                cj_half = []
                for h in range(2):
                    ch = prolB.tile([L, CTX // 2], F32, tag="cj_in", name="cj_in", bufs=1)
                    nc.sync.dma_start(out=ch, in_=ctx_d.ap()[:, h * 384:(h + 1) * 384])
                    cj_half.append(ch)
                for j in range(6):
                    tp = ps_tile(128, L)
                    src_h = cj_half[j // 3][:, (j % 3) * 128:(j % 3 + 1) * 128]
                    nc.tensor.transpose(tp, src_h, id_sb[0:L, 0:L])
                    cj = prolB.tile([128, L], F32, tag="ctxt", name="ctxt", bufs=2)
                    nc.scalar.activation(cj, tp, AF.Copy)
                    wkv_j = prolB.tile([128, KV_COLS], F32, tag="wkv", name="wkv", bufs=2)
                    nc.sync.dma_start(out=wkv_j, in_=wkvt_d.ap()[j * 128:(j + 1) * 128, :])
                    for m in range(6):
                        nc.tensor.matmul(kv_ps[m], wkv_j[:, m * 128:(m + 1) * 128], cj,
                                         start=(j == 0), stop=(j == 5))
                for i, (c0, cs) in enumerate(CCH):
                    nc.sync.dma_start(out=wqo_sb[i], in_=wqo_d.ap()[c0:c0 + cs, :])
                for m in range(6):
                    if m < 3:
                        c0, cs = CCH[m]
                        nc.scalar.activation(k_sb[m], kv_ps[m][0:cs, :], AF.Identity,
                                             bias=b6_sb[m][:, BK:BK + 1])
                    else:
                        c0, cs = CCH[m - 3]
                        nc.scalar.activation(v_sb[m - 3], kv_ps[m][0:cs, :], AF.Identity,
                                             bias=b6_sb[m - 3][:, BV:BV + 1])

            # kq[m] = sum_o (Wq*s)[o, m-chunk]^T k[o, :]   -> [ms, 77]
            for m, (m0, ms) in enumerate(CCH):
                kp = ps_tile(ms, L)
                for ki in range(3):
                    nc.tensor.matmul(kp, wqn_sb[ki][:, m0:m0 + ms], k_sb[ki],
                                     start=(ki == 0), stop=(ki == 2))
                nc.scalar.activation(kq_sb[m], kp, AF.Copy)
            # v2t = (Wo v)^T = v^T Wo^T  -> [77, 320]
            vp = ps_tile(L, C)
            for ki in range(3):
                nc.tensor.matmul(vp, v_sb[ki], wot_sb[ki],
                                 start=(ki == 0), stop=(ki == 2))
            nc.scalar.activation(v2t_sb, vp, AF.Copy)
            # blk = k^T (bq*s)  -> [77, 1]
            bp = ps_tile(L, 1)
            for ki in range(3):
                nc.tensor.matmul(bp, k_sb[ki], bqr_sb[ki],
                                 start=(ki == 0), stop=(ki == 2))
            nc.scalar.activation(blk_sb, bp, AF.Copy)

        # one-time consts for the rep body
        nc.sync.dma_start(out=ones_sb, in_=ones77_d.ap())
        nc.sync.dma_start(out=gmt_sb, in_=gmapt_d.ap())
        sm = ctx_stack.enter_context(tc.tile_pool(name="sm", bufs=1))
        for _rep in range(reps):
            s0 = 0
            for w in X_WIDTHS:
                ssl = slice(s0, s0 + w)
                for i, (c0, cs) in enumerate(CCH):
                    nc.sync.dma_start(out=xv(i, ssl), in_=x_d.ap()[c0:c0 + cs, ssl])
                s0 += w

            # ---------------- main loop (software-pipelined, 4 stages) ---------
            # Per tile t: FRONT = scores+exp (PE/ACT), MID = recip+mul
            # (DVE/Pool) one tile behind, TAIL = o-matmul+residual+copy
            # (PE/ACT) two tiles behind, BNSTATS (DVE) three tiles behind;
            # colsum (PE) is emitted last in each iteration.  The stage skew
            # keeps every engine's in-order queue fed with ready instructions.
            st_sb = [stats.tile([cs, NT, 6], F32, tag=f"st{i}", name=f"st{i}") for i, (c0, cs) in enumerate(CCH)]
            u_t, cb_t, rb_t = {}, {}, {}

            def front(it):
                nsl = slice(it * N_TILE, (it + 1) * N_TILE)
                # scores [77, n] = sum_k kq[k].T @ x[k]  (q-proj folded into kq)
                sp = ps_tile(L, N_TILE)
                for ki in range(3):
                    nc.tensor.matmul(sp, kq_sb[ki], xv(ki, nsl),
                                     start=(ki == 0), stop=(ki == 2))
                u = loop.tile([L, N_TILE], F32R, tag="u", name="u", bufs=3)
                nc.scalar.activation(u, sp, AF.Exp, bias=blk_sb)
                u_t[it] = u

            def colsum(it):
                # colsum of u broadcast to 77 partitions via all-ones lhsT
                cb = ps_tile(L, N_TILE)
                nc.tensor.matmul(cb, ones_sb, u_t[it], start=True, stop=True)
                cb_t[it] = cb

            def mid(it):
                rb = loop.tile([L, N_TILE], F32, tag="rb", name="rb", bufs=2)
                nc.vector.reciprocal(out=rb, in_=cb_t.pop(it))
                nc.gpsimd.tensor_mul(u_t[it], u_t[it].bitcast(F32), rb)

            def tail(it):
                nsl = slice(it * N_TILE, (it + 1) * N_TILE)
                u = u_t.pop(it)
                # xr[m] = v2t[:, m-chunk].T @ u + x[m]  (residual accumulated
                # on PE via identity matmul); ACT copies PSUM->SBUF in place.
                ops = []
                for m, (m0, ms) in enumerate(CCH):
                    op = ps_tile(ms, N_TILE)
                    nc.tensor.matmul(op, v2t_sb[:, m0:m0 + ms], u,
                                     start=True, stop=False)
                    nc.tensor.matmul(op, idr_sb[0:ms, 0:ms], xv(m, nsl),
                                     start=False, stop=True)
                    ops.append(op)
                for m, (m0, ms) in enumerate(CCH):
                    nc.scalar.activation(xb[m][:, nsl], ops[m], AF.Copy)

            def bnstats(it):
                nsl = slice(it * N_TILE, (it + 1) * N_TILE)
                for m, (m0, ms) in enumerate(CCH):
                    nc.vector.bn_stats(out=st_sb[m][:, it, :],
                                       in_=xv(m, nsl).bitcast(F32))

            for it in range(NT + 3):
                if it < NT:
                    front(it)
                if 0 <= it - 2 < NT:
                    tail(it - 2)
                if 0 <= it - 1 < NT:
                    mid(it - 1)
                if it < NT:
                    colsum(it)
                if 0 <= it - 3 < NT:
                    bnstats(it - 3)

            # ---------------- groupnorm stats ----------------
            st3 = []
            for i, (c0, cs) in enumerate(CCH):
                mv = sm.tile([cs, 2], F32, tag=f"mv{i}", name=f"mv{i}")
                nc.vector.bn_aggr(out=mv, in_=st_sb[i])
                s3 = sm.tile([cs, 3], F32, tag=f"s3{i}", name=f"s3{i}")
                # mean' = mean + bo ; var ; mean'^2
                nc.vector.tensor_add(s3[:, 0:1], mv[:, 0:1], b6_sb[i][:, BO:BO + 1])
                nc.vector.tensor_copy(s3[:, 1:2], mv[:, 1:2])
                nc.vector.tensor_mul(s3[:, 2:3], s3[:, 0:1], s3[:, 0:1])
                st3.append(s3)
            gp = ps_tile(G, 3)
            for i in range(3):
                nc.tensor.matmul(gp, gm_sb[i], st3[i], start=(i == 0), stop=(i == 2))
            # group stats: mu = s_mean/GS ; var = (s_var + s_mean2)/GS - mu^2
            gs = sm.tile([G, 3], F32, tag="gs", name="gs")
            nc.scalar.activation(gs, gp, AF.Copy)
            mu = sm.tile([G, 1], F32, tag="mu", name="mu")
            nc.scalar.activation(mu, gs[:, 0:1], AF.Copy, scale=1.0 / GS)
            tvar = sm.tile([G, 1], F32, tag="tvar", name="tvar")
            nc.vector.tensor_add(tvar, gs[:, 1:2], gs[:, 2:3])
            mu2 = sm.tile([G, 1], F32, tag="mu2", name="mu2")
            nc.vector.tensor_mul(mu2, mu, mu)
            var = sm.tile([G, 1], F32, tag="var", name="var")
            nc.vector.scalar_tensor_tensor(
                out=var, in0=tvar, scalar=1.0 / GS, in1=mu2,
                op0=ALU.mult, op1=ALU.subtract)
            # rstd = 1/sqrt(var + eps) on DVE via the bit-trick + 2 Newton
            # iterations (avoids swapping the ACT table away from Exp)
            I32 = mybir.dt.int32
            vpe = sm.tile([G, 1], F32, tag="vpe", name="vpe")
            nc.vector.tensor_scalar_add(vpe, var, EPS)
            y0i = sm.tile([G, 1], I32, tag="y0i", name="y0i")
            nc.vector.tensor_scalar(out=y0i, in0=vpe.bitcast(I32), scalar1=1,
                                    scalar2=None, op0=ALU.arith_shift_right)
            nc.vector.tensor_sub(y0i, cmagic_sb, y0i)
            rstd = sm.tile([G, 1], F32, tag="rstd", name="rstd")
            nc.vector.tensor_copy(rstd, y0i.bitcast(F32))
            yt = sm.tile([G, 1], F32, tag="yt", name="yt")
            for _nw in range(2):
                nc.vector.tensor_mul(yt, rstd, rstd)
                nc.vector.tensor_mul(yt, yt, vpe)
                nc.vector.scalar_tensor_tensor(
                    out=yt, in0=yt, scalar=-0.5, in1=c15_sb,
                    op0=ALU.mult, op1=ALU.add)
                nc.vector.tensor_mul(rstd, rstd, yt)
            mr = sm.tile([G, 2], F32, tag="mr", name="mr")
            nc.vector.tensor_copy(mr[:, 0:1], mu)
            nc.vector.tensor_copy(mr[:, 1:2], rstd)
            # broadcast back per channel: [cs, 2] = gmapT[m].T @ mr
            ab = []
            for m, (m0, ms) in enumerate(CCH):
                bp = ps_tile(ms, 2)
                nc.tensor.matmul(bp, gmt_sb[:, m0:m0 + ms], mr, start=True, stop=True)
                a_m = sm.tile([ms, 1], F32, tag=f"a{m}", name=f"a{m}")
                nc.vector.tensor_mul(a_m, bp[:, 1:2], b6_sb[m][:, GA:GA + 1])
                # b = beta + a*(bo - mu)
                t1 = sm.tile([ms, 1], F32, tag=f"t1{m}", name=f"t1{m}")
                nc.vector.tensor_sub(t1, b6_sb[m][:, BO:BO + 1], bp[:, 0:1])
                t2 = sm.tile([ms, 1], F32, tag=f"t2{m}", name=f"t2{m}")
                nc.vector.tensor_mul(t2, t1, a_m)
                b_m = sm.tile([ms, 1], F32, tag=f"b{m}", name=f"b{m}")
                nc.vector.tensor_add(b_m, b6_sb[m][:, BE:BE + 1], t2)
                ab.append((a_m, b_m))

            # ---------------- pass 2: swish + store (bf16 out halves DMA) -------
            BF16 = mybir.dt.bfloat16
            for s0 in range(0, N, O_SLICE):
                ssl = slice(s0, s0 + O_SLICE)
                for m, (m0, ms) in enumerate(CCH):
                    a_m, b_m = ab[m]
                    xmv = xv(m, ssl).bitcast(F32)
                    ob = loop.tile([ms, O_SLICE], BF16, tag="ob", name="ob", bufs=3)
                    if USE_SILU:
                        nc.scalar.activation(ob, xmv, AF.Silu, bias=b_m, scale=a_m)
                    else:
                        # sim fallback: xn*sigmoid(xn) with xn = a*x + b, via
                        #   sig = sigmoid(a*x+b); t = (x*a)*sig; out = (sig*b) + t
                        sgm = loop.tile([ms, O_SLICE], F32, tag="sg", name="sg", bufs=1)
                        nc.scalar.activation(sgm, xmv, AF.Sigmoid, bias=b_m, scale=a_m)
                        nc.vector.scalar_tensor_tensor(
                            out=xmv, in0=xmv, scalar=a_m, in1=sgm,
                            op0=ALU.mult, op1=ALU.mult)
                        nc.vector.scalar_tensor_tensor(
                            out=xmv, in0=sgm, scalar=b_m, in1=xmv,
                            op0=ALU.mult, op1=ALU.add)
                        nc.vector.tensor_copy(ob, xmv)
                    nc.sync.dma_start(out=out_d.ap()[m0:m0 + ms, ssl], in_=ob)


_NC_CACHE = None


def _get_nc():
    global _NC_CACHE
    if _NC_CACHE is None:
        _NC_CACHE = _build()
    return _NC_CACHE


def _host_consts(Wq, bq, Wk, bk, Wv, bv, Wo, bo, gamma, beta):
    s = float(C) ** -0.5
    wqo = np.concatenate([(Wq * s).astype(np.float32),
                          Wo.T.astype(np.float32)], axis=1)
    wqo = np.ascontiguousarray(wqo)
    wkvt = np.zeros((CTX, KV_COLS), np.float32)
    wkt = Wk.T.astype(np.float32)   # [CTX, C]
    wvt = Wv.T.astype(np.float32)
    wkvt[:, 0:128] = wkt[:, 0:128]
    wkvt[:, 128:256] = wkt[:, 128:256]
    wkvt[:, 256:320] = wkt[:, 256:320]
    wkvt[:, 384:512] = wvt[:, 0:128]
    wkvt[:, 512:640] = wvt[:, 128:256]
    wkvt[:, 640:704] = wvt[:, 256:320]
    bias6 = np.stack([bq * s, bk, bv, bo, gamma, beta], axis=1).astype(np.float32)
    gmap = np.zeros((C, G), np.float32)
    gmap[np.arange(C), np.arange(C) // GS] = 1.0
    # per-channel consts [bias6 | bq*s | gmap], packed per chunk into cpack
    cpk = np.concatenate(
        [bias6, (bq * s).astype(np.float32).reshape(C, 1), gmap], axis=1)
    NCP = 7 + G
    cpack = np.zeros((128, 128 + 3 * NCP + 5), np.float32)
    cpack[:, 0:128] = np.eye(128, dtype=np.float32)
    for i, (c0, cs) in enumerate(CCH):
        cpack[0:cs, 128 + i * NCP:128 + (i + 1) * NCP] = cpk[c0:c0 + cs, :]
        cpack[0:cs, 128 + 3 * NCP + i] = (bq[c0:c0 + cs] * s).astype(np.float32)
    cpack[:, 128 + 3 * NCP + 3] = np.float32(
        np.frombuffer(np.uint32(0x5F3759DF).tobytes(), np.float32)[0])
    cpack[:, 128 + 3 * NCP + 4] = 1.5
    gmapt = np.ascontiguousarray(gmap.T)
    ones77 = np.ones((L, L), np.float32)
    ident = np.eye(128, dtype=np.float32)
    return dict(wqo=wqo, wkvt=wkvt, cpack=np.ascontiguousarray(cpack),
                gmapt=gmapt, ones77=ones77, identr=ident)


def kernel(x, context, Wq, bq, Wk, bk, Wv, bv, Wo, bo, gamma, beta,
           _return_results=False, _trace=False):
    x = np.asarray(x, np.float32)
    context = np.asarray(context, np.float32)
    consts = _host_consts(np.asarray(Wq, np.float32), np.asarray(bq, np.float32),
                          np.asarray(Wk, np.float32), np.asarray(bk, np.float32),
                          np.asarray(Wv, np.float32), np.asarray(bv, np.float32),
                          np.asarray(Wo, np.float32), np.asarray(bo, np.float32),
                          np.asarray(gamma, np.float32), np.asarray(beta, np.float32))
    nc = _get_nc()
    in_maps = []
    for b in range(B):
        m = dict(consts)
        m["x"] = np.ascontiguousarray(x[b].reshape(C, N))
        m["ctx"] = np.ascontiguousarray(context[b])
        in_maps.append(m)
    res = run_bass_kernel_spmd(nc, in_maps, core_ids=list(range(B)), trace=_trace)
    out = np.stack([np.asarray(res.results[b]["out"], dtype=np.float32)
                    .reshape(C, D, D, D) for b in range(B)])
    if _return_results:
        return out, res
    return out



# revision 46
# speedup vs baseline: 74.3694x; 8.7861x over previous
"""Trainium2 Bass kernel for CrossAttention + GroupNorm + Swish (nn_CrossAttention).

Reference computation (per batch element b, xf = x[b] reshaped [C, N]):
    q  = Wq @ xf + bq                       [C, N]
    k  = Wk @ ctx^T + bk                    [C, L]
    v  = Wv @ ctx^T + bv                    [C, L]
    qk = (q^T k) * C^-0.5                   [N, L]
    w  = softmax(qk, axis=-1)
    h  = v @ w^T                            [C, N]
    o  = Wo @ h + bo
    xr = o + xf
    out = swish(groupnorm(xr; 32 groups over (C/32, N)) * gamma + beta)

Sharding: data-parallel over batch B=8 across the 8 NeuronCores (no collectives).

Key algebraic restructuring (L=77 << C=320 makes attention low-rank):
    scores^T = k'^T xf + blk      with k'  = (Wq*s)^T k   [C, L]   (one-time)
                                       blk = k^T (bq*s)   [L, 1]   (one-time,
                                       applied as per-partition bias in Exp)
    o        = v2t^T w            with v2t = (Wo v)^T     [L, C]   (one-time)
so the per-tile work is only: 3 score matmuls, Exp, ones-matmul colsum,
reciprocal, mul, 3 o-matmuls, residual add, bn_stats.  The q/o projections
(18 matmuls + 6 ACT ops per tile in the direct form) disappear.

Device algorithm (per core):
  - x chunks resident in SBUF ([128|128|64] x 13824 fp32r), loaded in 21
    large DMAs (two small leading slices for a fast ramp), updated in place
    with xr, stored as bf16 after pass 2 (tolerance 2e-2 >> bf16 rounding).
  - n-tiles of 512 (27 tiles); main-loop matmuls fp32r (full PE rate at
    moving dim 512); tiny prologue matmuls plain fp32 (fp32r has ISA
    restrictions at odd/small moving dims).
  - the residual add rides the PE: the o-matmul PSUM group accumulates an
    identity matmul of x, so ACT's PSUM->SBUF copy IS the residual write;
    DVE only does softmax reciprocal + bn_stats, Pool does the softmax mul.
  - main loop is software-pipelined 4 stages deep (front: scores+exp,
    mid(-1): recip+mul, tail(-2): o-matmul+copy, bn(-3)) so every engine's
    in-order queue only sees ready instructions.
  - softmax without max-subtraction (scores tiny; scale folded into k').
  - colsum of exp-scores broadcast across partitions via one all-ones matmul.
  - GroupNorm stats via bn_stats/bn_aggr per channel + group-membership
    matmuls (gmap [C,32], gmapT [32,C]); bias bo folded analytically;
    1/sqrt(var+eps) via DVE bit-trick + Newton (avoids an ACT table swap).
  - pass 2: out = Silu(a_c * xr + b_c) as one ACT op per [cs, 1728] slice,
    written to bf16 staging tiles and DMA'd out (half the store traffic).
  - consts are packed ([ident|bias6|bq*s|gmap|rsqrt magics] in one tensor,
    Wq*s and Wo^T concatenated) to minimize serialized HWDGE descriptor time
    during the ramp.
"""
import sys

sys.path.insert(0, "/opt/trn_rl_repo")

import numpy as np

import concourse.tile as tile
from concourse import bacc, mybir
from concourse.bass_utils import run_bass_kernel_spmd

F32 = mybir.dt.float32
F32R = mybir.dt.float32r
BF16 = mybir.dt.bfloat16
AF = mybir.ActivationFunctionType
ALU = mybir.AluOpType

# Problem shapes (hardcoded; harness contract)
B, C, D, L, CTX = 8, 320, 24, 77, 768
N = D * D * D            # 13824 spatial positions
G = 32                   # groupnorm groups
GS = C // G              # 10 channels per group
EPS = 1e-5
N_TILE = 512
NT = N // N_TILE         # 27
CCH = [(0, 128), (128, 128), (256, 64)]   # channel chunks (start, size)
KV_COLS = 768            # padded concat [k0,k1,k2+pad, v0,v1,v2+pad]
X_WIDTHS = [1152, 1152] + [2304] * 5   # x load slices (21 DMAs, fast start)
O_SLICE = 1728           # pass-2 silu/store granularity (24 ACT ops / DMAs)

# Silu isn't implemented in CoreSim; flip for simulation runs.
USE_SILU = True

# bias6 columns
BQ, BK, BV, BO, GA, BE = range(6)


def _build(reps=1):
    nc = bacc.Bacc(trn_type="TRN2", target_bir_lowering=False, debug=False)

    x_d = nc.dram_tensor("x", [C, N], F32R, kind="ExternalInput")
    ctx_d = nc.dram_tensor("ctx", [L, CTX], F32, kind="ExternalInput")
    wqo_d = nc.dram_tensor("wqo", [C, 2 * C], F32, kind="ExternalInput")
    wkvt_d = nc.dram_tensor("wkvt", [CTX, KV_COLS], F32, kind="ExternalInput")
    # one packed f32 const tensor: [ident(128) | cpk0|cpk1|cpk2 (39 each) | bqr0..2]
    cpack_d = nc.dram_tensor("cpack", [128, 128 + 3 * (7 + G) + 5], F32,
                             kind="ExternalInput")
    gmapt_d = nc.dram_tensor("gmapt", [G, C], F32, kind="ExternalInput")
    ones77_d = nc.dram_tensor("ones77", [L, L], F32R, kind="ExternalInput")
    identr_d = nc.dram_tensor("identr", [128, 128], F32R, kind="ExternalInput")
    out_d = nc.dram_tensor("out", [C, N], mybir.dt.bfloat16, kind="ExternalOutput")

    with tile.TileContext(nc) as tc:
        _emit(nc, tc, x_d, ctx_d, wqo_d, wkvt_d, cpack_d,
              gmapt_d, ones77_d, identr_d, out_d, reps)
    nc.compile()
    return nc


def _emit(nc, tc, x_d, ctx_d, wqo_d, wkvt_d, cpack_d,
          gmapt_d, ones77_d, identr_d, out_d, reps=1):
    from contextlib import ExitStack

    with ExitStack() as ctx_stack:
        const = ctx_stack.enter_context(tc.tile_pool(name="const", bufs=1))
        xpool = ctx_stack.enter_context(tc.tile_pool(name="xbuf", bufs=1))
        kvres = ctx_stack.enter_context(tc.tile_pool(name="kvres", bufs=1))
        loop = ctx_stack.enter_context(tc.tile_pool(name="loop", bufs=2))
        stats = ctx_stack.enter_context(tc.tile_pool(name="stats", bufs=1))

        psum = ctx_stack.enter_context(tc.tile_pool(name="psum", bufs=8, space="PSUM"))

        def ps_tile(p, f):
            return psum.tile([p, f], F32, tag="mm", name="mm")

        # ---------------- constants ----------------
        # DMA issue order is tuned for ramp time: ident + packed consts first
        # (prologue-critical), ctx/wkv/wqn/wot inside the prologue, then
        # ones77, then the 18 big x slices, then gmapt (needed only at stats).
        NCP = 7 + G
        cpack_sb = const.tile([128, 128 + 3 * NCP + 5], F32, tag="cpack", name="cpack")
        gmt_sb = const.tile([G, C], F32, tag="gmt", name="gmt")
        ones_sb = const.tile([L, L], F32R, tag="ones77", name="ones77")
        id_sb = cpack_sb[:, 0:128]
        cpk_sb = [cpack_sb[0:cs, 128 + i * NCP:128 + (i + 1) * NCP]
                  for i, (c0, cs) in enumerate(CCH)]
        b6_sb = cpk_sb                       # cols 0..5 = bq*s|bk|bv|bo|gamma|beta
        gm_sb = [t[:, 7:7 + G] for t in cpk_sb]
        bqr_sb = [cpack_sb[0:cs, 128 + 3 * NCP + i:128 + 3 * NCP + i + 1]
                  for i, (c0, cs) in enumerate(CCH)]
        cmagic_sb = cpack_sb[0:G, 128 + 3 * NCP + 3:128 + 3 * NCP + 4].bitcast(mybir.dt.int32)
        c15_sb = cpack_sb[0:G, 128 + 3 * NCP + 4:128 + 3 * NCP + 5]
        idr_sb = const.tile([128, 128], F32R, tag="identr", name="identr")
        nc.sync.dma_start(out=cpack_sb, in_=cpack_d.ap())
        nc.sync.dma_start(out=idr_sb, in_=identr_d.ap())

        # x resident chunks (loaded below, after the prologue's DMAs are queued)
        xb = [xpool.tile([cs, N], F32R, tag=f"xb{i}", name=f"xb{i}")
              for i, (c0, cs) in enumerate(CCH)]

        def xv(m, sl):
            return xb[m][:, sl]

        # ---------------- prologue: k, v -> k' (kq), v2t, blk ----------------
        kq_sb = [kvres.tile([cs, L], F32R, tag=f"kq{i}", name=f"kq{i}") for i, (c0, cs) in enumerate(CCH)]
        v2t_sb = kvres.tile([L, C], F32R, tag="v2t", name="v2t")
        blk_sb = kvres.tile([L, 1], F32, tag="blk", name="blk")

        with tc.tile_pool(name="prolA", bufs=1) as prolA:
            # k/v and the fused-projection weights span both prologue phases
            k_sb = [prolA.tile([cs, L], F32, tag=f"k{i}", name=f"k{i}")
                    for i, (c0, cs) in enumerate(CCH)]
            v_sb = [prolA.tile([cs, L], F32, tag=f"v{i}", name=f"v{i}")
                    for i, (c0, cs) in enumerate(CCH)]
            wqo_sb = [prolA.tile([cs, 2 * C], F32, tag=f"wqo{i}", name=f"wqo{i}")
                      for i, (c0, cs) in enumerate(CCH)]
            wqn_sb = [t[:, 0:C] for t in wqo_sb]
            wot_sb = [t[:, C:2 * C] for t in wqo_sb]

            with tc.tile_pool(name="prolB", bufs=1) as prolB:
                kv_ps = [ps_tile(128, L) for _ in range(6)]
                cj_half = []
                for h in range(2):
                    ch = prolB.tile([L, CTX // 2], F32, tag="cj_in", name="cj_in", bufs=1)
                    # issue on the (otherwise idle) GpSimd queue: the h=1
                    # DMA's write-after-read wait on the shared buffer must
                    # not block the SP queue feeding wkv/wqo/x right behind
                    nc.gpsimd.dma_start(out=ch, in_=ctx_d.ap()[:, h * 384:(h + 1) * 384])
                    cj_half.append(ch)
                for j in range(6):
                    tp = ps_tile(128, L)
                    src_h = cj_half[j // 3][:, (j % 3) * 128:(j % 3 + 1) * 128]
                    nc.tensor.transpose(tp, src_h, id_sb[0:L, 0:L])
                    cj = prolB.tile([128, L], F32, tag="ctxt", name="ctxt", bufs=2)
                    nc.scalar.activation(cj, tp, AF.Copy)
                    wkv_j = prolB.tile([128, KV_COLS], F32, tag="wkv", name="wkv", bufs=2)
                    nc.sync.dma_start(out=wkv_j, in_=wkvt_d.ap()[j * 128:(j + 1) * 128, :])
                    for m in range(6):
                        nc.tensor.matmul(kv_ps[m], wkv_j[:, m * 128:(m + 1) * 128], cj,
                                         start=(j == 0), stop=(j == 5))
                for i, (c0, cs) in enumerate(CCH):
                    # ACT queue: skips past the SP queue, which creeps in
                    # lock-step with the kv loop on wkv buffer-reuse waits
                    nc.scalar.dma_start(out=wqo_sb[i], in_=wqo_d.ap()[c0:c0 + cs, :])
                for m in range(6):
                    if m < 3:
                        c0, cs = CCH[m]
                        nc.scalar.activation(k_sb[m], kv_ps[m][0:cs, :], AF.Identity,
                                             bias=b6_sb[m][:, BK:BK + 1])
                    else:
                        c0, cs = CCH[m - 3]
                        nc.scalar.activation(v_sb[m - 3], kv_ps[m][0:cs, :], AF.Identity,
                                             bias=b6_sb[m - 3][:, BV:BV + 1])

            # kq[m] = sum_o (Wq*s)[o, m-chunk]^T k[o, :]   -> [ms, 77]
            for m, (m0, ms) in enumerate(CCH):
                kp = ps_tile(ms, L)
                for ki in range(3):
                    nc.tensor.matmul(kp, wqn_sb[ki][:, m0:m0 + ms], k_sb[ki],
                                     start=(ki == 0), stop=(ki == 2))
                nc.scalar.activation(kq_sb[m], kp, AF.Copy)
            # v2t = (Wo v)^T = v^T Wo^T  -> [77, 320]
            vp = ps_tile(L, C)
            for ki in range(3):
                nc.tensor.matmul(vp, v_sb[ki], wot_sb[ki],
                                 start=(ki == 0), stop=(ki == 2))
            nc.scalar.activation(v2t_sb, vp, AF.Copy)
            # blk = k^T (bq*s)  -> [77, 1]
            bp = ps_tile(L, 1)
            for ki in range(3):
                nc.tensor.matmul(bp, k_sb[ki], bqr_sb[ki],
                                 start=(ki == 0), stop=(ki == 2))
            nc.scalar.activation(blk_sb, bp, AF.Copy)

        # one-time consts for the rep body
        nc.sync.dma_start(out=ones_sb, in_=ones77_d.ap())
        nc.sync.dma_start(out=gmt_sb, in_=gmapt_d.ap())
        sm = ctx_stack.enter_context(tc.tile_pool(name="sm", bufs=1))
        for _rep in range(reps):
            s0 = 0
            for w in X_WIDTHS:
                ssl = slice(s0, s0 + w)
                for i, (c0, cs) in enumerate(CCH):
                    nc.sync.dma_start(out=xv(i, ssl), in_=x_d.ap()[c0:c0 + cs, ssl])
                s0 += w

            # ---------------- main loop (software-pipelined, 4 stages) ---------
            # Per tile t: FRONT = scores+exp (PE/ACT), MID = recip+mul
            # (DVE/Pool) one tile behind, TAIL = o-matmul+residual+copy
            # (PE/ACT) two tiles behind, BNSTATS (DVE) three tiles behind;
            # colsum (PE) is emitted last in each iteration.  The stage skew
            # keeps every engine's in-order queue fed with ready instructions.
            st_sb = [stats.tile([cs, NT, 6], F32, tag=f"st{i}", name=f"st{i}") for i, (c0, cs) in enumerate(CCH)]
            u_t, cb_t, rb_t = {}, {}, {}

            def front(it):
                nsl = slice(it * N_TILE, (it + 1) * N_TILE)
                # scores [77, n] = sum_k kq[k].T @ x[k]  (q-proj folded into kq)
                sp = ps_tile(L, N_TILE)
                for ki in range(3):
                    nc.tensor.matmul(sp, kq_sb[ki], xv(ki, nsl),
                                     start=(ki == 0), stop=(ki == 2))
                u = loop.tile([L, N_TILE], F32R, tag="u", name="u", bufs=3)
                nc.scalar.activation(u, sp, AF.Exp, bias=blk_sb)
                u_t[it] = u

            def colsum(it):
                # colsum of u broadcast to 77 partitions via all-ones lhsT
                cb = ps_tile(L, N_TILE)
                nc.tensor.matmul(cb, ones_sb, u_t[it], start=True, stop=True)
                cb_t[it] = cb

            def mid(it):
                rb = loop.tile([L, N_TILE], F32, tag="rb", name="rb", bufs=2)
                nc.vector.reciprocal(out=rb, in_=cb_t.pop(it))
                nc.gpsimd.tensor_mul(u_t[it], u_t[it].bitcast(F32), rb)

            def tail(it):
                nsl = slice(it * N_TILE, (it + 1) * N_TILE)
                u = u_t.pop(it)
                # xr[m] = v2t[:, m-chunk].T @ u + x[m]  (residual accumulated
                # on PE via identity matmul); ACT copies PSUM->SBUF in place.
                ops = []
                for m, (m0, ms) in enumerate(CCH):
                    op = ps_tile(ms, N_TILE)
                    nc.tensor.matmul(op, v2t_sb[:, m0:m0 + ms], u,
                                     start=True, stop=False)
                    nc.tensor.matmul(op, idr_sb[0:ms, 0:ms], xv(m, nsl),
                                     start=False, stop=True)
                    ops.append(op)
                for m, (m0, ms) in enumerate(CCH):
                    nc.scalar.activation(xb[m][:, nsl], ops[m], AF.Copy)

            def bnstats(it):
                nsl = slice(it * N_TILE, (it + 1) * N_TILE)
                for m, (m0, ms) in enumerate(CCH):
                    nc.vector.bn_stats(out=st_sb[m][:, it, :],
                                       in_=xv(m, nsl).bitcast(F32))

            for it in range(NT + 3):
                if it < NT:
                    front(it)
                if 0 <= it - 2 < NT:
                    tail(it - 2)
                if 0 <= it - 1 < NT:
                    mid(it - 1)
                if it < NT:
                    colsum(it)
                if 0 <= it - 3 < NT:
                    bnstats(it - 3)

            # ---------------- groupnorm stats ----------------
            st3 = []
            for i, (c0, cs) in enumerate(CCH):
                mv = sm.tile([cs, 2], F32, tag=f"mv{i}", name=f"mv{i}")
                nc.vector.bn_aggr(out=mv, in_=st_sb[i])
                s3 = sm.tile([cs, 3], F32, tag=f"s3{i}", name=f"s3{i}")
                # mean' = mean + bo ; var ; mean'^2
                nc.vector.tensor_add(s3[:, 0:1], mv[:, 0:1], b6_sb[i][:, BO:BO + 1])
                nc.vector.tensor_copy(s3[:, 1:2], mv[:, 1:2])
                nc.vector.tensor_mul(s3[:, 2:3], s3[:, 0:1], s3[:, 0:1])
                st3.append(s3)
            gp = ps_tile(G, 3)
            for i in range(3):
                nc.tensor.matmul(gp, gm_sb[i], st3[i], start=(i == 0), stop=(i == 2))
            # group stats: mu = s_mean/GS ; var = (s_var + s_mean2)/GS - mu^2
            gs = sm.tile([G, 3], F32, tag="gs", name="gs")
            nc.scalar.activation(gs, gp, AF.Copy)
            mu = sm.tile([G, 1], F32, tag="mu", name="mu")
            nc.scalar.activation(mu, gs[:, 0:1], AF.Copy, scale=1.0 / GS)
            tvar = sm.tile([G, 1], F32, tag="tvar", name="tvar")
            nc.vector.tensor_add(tvar, gs[:, 1:2], gs[:, 2:3])
            mu2 = sm.tile([G, 1], F32, tag="mu2", name="mu2")
            nc.vector.tensor_mul(mu2, mu, mu)
            var = sm.tile([G, 1], F32, tag="var", name="var")
            nc.vector.scalar_tensor_tensor(
                out=var, in0=tvar, scalar=1.0 / GS, in1=mu2,
                op0=ALU.mult, op1=ALU.subtract)
            # rstd = 1/sqrt(var + eps) on DVE via the bit-trick + 2 Newton
            # iterations (avoids swapping the ACT table away from Exp)
            I32 = mybir.dt.int32
            vpe = sm.tile([G, 1], F32, tag="vpe", name="vpe")
            nc.vector.tensor_scalar_add(vpe, var, EPS)
            y0i = sm.tile([G, 1], I32, tag="y0i", name="y0i")
            nc.vector.tensor_scalar(out=y0i, in0=vpe.bitcast(I32), scalar1=1,
                                    scalar2=None, op0=ALU.arith_shift_right)
            nc.vector.tensor_sub(y0i, cmagic_sb, y0i)
            rstd = sm.tile([G, 1], F32, tag="rstd", name="rstd")
            nc.vector.tensor_copy(rstd, y0i.bitcast(F32))
            yt = sm.tile([G, 1], F32, tag="yt", name="yt")
            for _nw in range(2):
                nc.vector.tensor_mul(yt, rstd, rstd)
                nc.vector.tensor_mul(yt, yt, vpe)
                nc.vector.scalar_tensor_tensor(
                    out=yt, in0=yt, scalar=-0.5, in1=c15_sb,
                    op0=ALU.mult, op1=ALU.add)
                nc.vector.tensor_mul(rstd, rstd, yt)
            mr = sm.tile([G, 2], F32, tag="mr", name="mr")
            nc.vector.tensor_copy(mr[:, 0:1], mu)
            nc.vector.tensor_copy(mr[:, 1:2], rstd)
            # broadcast back per channel: [cs, 2] = gmapT[m].T @ mr
            ab = []
            for m, (m0, ms) in enumerate(CCH):
                bp = ps_tile(ms, 2)
                nc.tensor.matmul(bp, gmt_sb[:, m0:m0 + ms], mr, start=True, stop=True)
                a_m = sm.tile([ms, 1], F32, tag=f"a{m}", name=f"a{m}")
                nc.vector.tensor_mul(a_m, bp[:, 1:2], b6_sb[m][:, GA:GA + 1])
                # b = beta + a*(bo - mu)
                t1 = sm.tile([ms, 1], F32, tag=f"t1{m}", name=f"t1{m}")
                nc.vector.tensor_sub(t1, b6_sb[m][:, BO:BO + 1], bp[:, 0:1])
                t2 = sm.tile([ms, 1], F32, tag=f"t2{m}", name=f"t2{m}")
                nc.vector.tensor_mul(t2, t1, a_m)
                b_m = sm.tile([ms, 1], F32, tag=f"b{m}", name=f"b{m}")
                nc.vector.tensor_add(b_m, b6_sb[m][:, BE:BE + 1], t2)
                ab.append((a_m, b_m))

            # ---------------- pass 2: swish + store (bf16 out halves DMA) -------
            for s0 in range(0, N, O_SLICE):
                ssl = slice(s0, s0 + O_SLICE)
                for m, (m0, ms) in enumerate(CCH):
                    a_m, b_m = ab[m]
                    xmv = xv(m, ssl).bitcast(F32)
                    ob = loop.tile([ms, O_SLICE], BF16, tag="ob", name="ob", bufs=3)
                    if USE_SILU:
                        nc.scalar.activation(ob, xmv, AF.Silu, bias=b_m, scale=a_m)
                    else:
                        # sim fallback: xn*sigmoid(xn) with xn = a*x + b, via
                        #   sig = sigmoid(a*x+b); t = (x*a)*sig; out = (sig*b) + t
                        sgm = loop.tile([ms, O_SLICE], F32, tag="sg", name="sg", bufs=1)
                        nc.scalar.activation(sgm, xmv, AF.Sigmoid, bias=b_m, scale=a_m)
                        nc.vector.scalar_tensor_tensor(
                            out=xmv, in0=xmv, scalar=a_m, in1=sgm,
                            op0=ALU.mult, op1=ALU.mult)
                        nc.vector.scalar_tensor_tensor(
                            out=xmv, in0=sgm, scalar=b_m, in1=xmv,
                            op0=ALU.mult, op1=ALU.add)
                        nc.vector.tensor_copy(ob, xmv)
                    nc.sync.dma_start(out=out_d.ap()[m0:m0 + ms, ssl], in_=ob)


_NC_CACHE = None


def _get_nc():
    global _NC_CACHE
    if _NC_CACHE is None:
        _NC_CACHE = _build()
    return _NC_CACHE


def _host_consts(Wq, bq, Wk, bk, Wv, bv, Wo, bo, gamma, beta):
    s = float(C) ** -0.5
    wqo = np.concatenate([(Wq * s).astype(np.float32),
                          Wo.T.astype(np.float32)], axis=1)
    wqo = np.ascontiguousarray(wqo)
    wkvt = np.zeros((CTX, KV_COLS), np.float32)
    wkt = Wk.T.astype(np.float32)   # [CTX, C]
    wvt = Wv.T.astype(np.float32)
    wkvt[:, 0:128] = wkt[:, 0:128]
    wkvt[:, 128:256] = wkt[:, 128:256]
    wkvt[:, 256:320] = wkt[:, 256:320]
    wkvt[:, 384:512] = wvt[:, 0:128]
    wkvt[:, 512:640] = wvt[:, 128:256]
    wkvt[:, 640:704] = wvt[:, 256:320]
    bias6 = np.stack([bq * s, bk, bv, bo, gamma, beta], axis=1).astype(np.float32)
    gmap = np.zeros((C, G), np.float32)
    gmap[np.arange(C), np.arange(C) // GS] = 1.0
    # per-channel consts [bias6 | bq*s | gmap], packed per chunk into cpack
    cpk = np.concatenate(
        [bias6, (bq * s).astype(np.float32).reshape(C, 1), gmap], axis=1)
    NCP = 7 + G
    cpack = np.zeros((128, 128 + 3 * NCP + 5), np.float32)
    cpack[:, 0:128] = np.eye(128, dtype=np.float32)
    for i, (c0, cs) in enumerate(CCH):
        cpack[0:cs, 128 + i * NCP:128 + (i + 1) * NCP] = cpk[c0:c0 + cs, :]
        cpack[0:cs, 128 + 3 * NCP + i] = (bq[c0:c0 + cs] * s).astype(np.float32)
    cpack[:, 128 + 3 * NCP + 3] = np.float32(
        np.frombuffer(np.uint32(0x5F3759DF).tobytes(), np.float32)[0])
    cpack[:, 128 + 3 * NCP + 4] = 1.5
    gmapt = np.ascontiguousarray(gmap.T)
    ones77 = np.ones((L, L), np.float32)
    ident = np.eye(128, dtype=np.float32)
    return dict(wqo=wqo, wkvt=wkvt, cpack=np.ascontiguousarray(cpack),
                gmapt=gmapt, ones77=ones77, identr=ident)


def kernel(x, context, Wq, bq, Wk, bk, Wv, bv, Wo, bo, gamma, beta,
           _return_results=False, _trace=False):
    x = np.asarray(x, np.float32)
    context = np.asarray(context, np.float32)
    consts = _host_consts(np.asarray(Wq, np.float32), np.asarray(bq, np.float32),
                          np.asarray(Wk, np.float32), np.asarray(bk, np.float32),
                          np.asarray(Wv, np.float32), np.asarray(bv, np.float32),
                          np.asarray(Wo, np.float32), np.asarray(bo, np.float32),
                          np.asarray(gamma, np.float32), np.asarray(beta, np.float32))
    nc = _get_nc()
    in_maps = []
    for b in range(B):
        m = dict(consts)
        m["x"] = np.ascontiguousarray(x[b].reshape(C, N))
        m["ctx"] = np.ascontiguousarray(context[b])
        in_maps.append(m)
    res = run_bass_kernel_spmd(nc, in_maps, core_ids=list(range(B)), trace=_trace)
    out = np.stack([np.asarray(res.results[b]["out"], dtype=np.float32)
                    .reshape(C, D, D, D) for b in range(B)])
    if _return_results:
        return out, res
    return out



# revision 50
# speedup vs baseline: 481.5258x; 6.4748x over previous
"""Trainium2 Bass kernel for CrossAttention + GroupNorm + Swish (nn_CrossAttention).

Reference computation (per batch element b, xf = x[b] reshaped [C, N]):
    q  = Wq @ xf + bq                       [C, N]
    k  = Wk @ ctx^T + bk                    [C, L]
    v  = Wv @ ctx^T + bv                    [C, L]
    qk = (q^T k) * C^-0.5                   [N, L]
    w  = softmax(qk, axis=-1)
    h  = v @ w^T                            [C, N]
    o  = Wo @ h + bo
    xr = o + xf
    out = swish(groupnorm(xr; 32 groups over (C/32, N)) * gamma + beta)

Sharding: data-parallel over batch B=8 across the 8 NeuronCores (no collectives).

Key algebraic restructuring (L=77 << C=320 makes attention low-rank):
    scores^T = k'^T xf + blk      with k'  = (Wq*s)^T k   [C, L]   (one-time)
                                       blk = k^T (bq*s)   [L, 1]   (one-time,
                                       applied as per-partition bias in Exp)
    o        = v2t^T w            with v2t = (Wo v)^T     [L, C]   (one-time)
so the per-tile work is only: 3 score matmuls, Exp, ones-matmul colsum,
reciprocal, mul, 3 o-matmuls, residual add, bn_stats.  The q/o projections
(18 matmuls + 6 ACT ops per tile in the direct form) disappear.

Device algorithm (per core):
  - x chunks resident in SBUF ([128|128|64] x 13824 fp32r), loaded in 21
    large DMAs (two small leading slices for a fast ramp), updated in place
    with xr, stored as bf16 after pass 2 (tolerance 2e-2 >> bf16 rounding).
  - n-tiles of 512 (27 tiles); main-loop matmuls fp32r (full PE rate at
    moving dim 512); tiny prologue matmuls plain fp32 (fp32r has ISA
    restrictions at odd/small moving dims).
  - the residual add rides the PE: the o-matmul PSUM group accumulates an
    identity matmul of x, so ACT's PSUM->SBUF copy IS the residual write;
    DVE only does softmax reciprocal + bn_stats, Pool does the softmax mul.
  - main loop is software-pipelined 4 stages deep (front: scores+exp,
    mid(-1): recip+mul, tail(-2): o-matmul+copy, bn(-3)) so every engine's
    in-order queue only sees ready instructions.
  - softmax without max-subtraction (scores tiny; scale folded into k').
  - colsum of exp-scores broadcast across partitions via one all-ones matmul.
  - GroupNorm stats via bn_stats/bn_aggr per channel + group-membership
    matmuls (gmap [C,32], gmapT [32,C]); bias bo folded analytically;
    1/sqrt(var+eps) via DVE bit-trick + Newton (avoids an ACT table swap).
  - pass 2: out = Silu(a_c * xr + b_c) as one ACT op per [cs, 1728] slice,
    written to bf16 staging tiles and DMA'd out (half the store traffic).
  - consts are packed ([ident|bias6|bq*s|gmap|rsqrt magics] in one tensor,
    Wq*s and Wo^T concatenated) to minimize serialized HWDGE descriptor time
    during the ramp.
"""
import sys

sys.path.insert(0, "/opt/trn_rl_repo")

import numpy as np

import concourse.tile as tile
from concourse import bacc, mybir
from concourse.bass_utils import run_bass_kernel_spmd

F32 = mybir.dt.float32
F32R = mybir.dt.float32r
BF16 = mybir.dt.bfloat16
AF = mybir.ActivationFunctionType
ALU = mybir.AluOpType

# Problem shapes (hardcoded; harness contract)
B, C, D, L, CTX = 8, 320, 24, 77, 768
N = D * D * D            # 13824 spatial positions
G = 32                   # groupnorm groups
GS = C // G              # 10 channels per group
EPS = 1e-5
N_TILE = 512
NT = N // N_TILE         # 27
CCH = [(0, 128), (128, 128), (256, 64)]   # channel chunks (start, size)
KV_COLS = 640            # packed concat [k(320) | v(320)]
KV_OFF = [0, 128, 256, 320, 448, 576]   # block offsets in wkvt cols
KV_SZ = [128, 128, 64, 128, 128, 64]
X_WIDTHS = [1152, 1152] + [2304] * 5   # x load slices (21 DMAs, fast start)
O_SLICE = 1728           # pass-2 silu/store granularity (24 ACT ops / DMAs)

# Silu isn't implemented in CoreSim; flip for simulation runs.
USE_SILU = True

# bias6 columns
BQ, BK, BV, BO, GA, BE = range(6)


def _build(reps=1):
    nc = bacc.Bacc(trn_type="TRN2", target_bir_lowering=False, debug=False)

    x_d = nc.dram_tensor("x", [C, N], F32R, kind="ExternalInput")
    ctx_d = nc.dram_tensor("ctx", [L, CTX], F32, kind="ExternalInput")
    wqo_d = nc.dram_tensor("wqo", [C, 2 * C], F32, kind="ExternalInput")
    wkvt_d = nc.dram_tensor("wkvt", [CTX, KV_COLS], F32, kind="ExternalInput")
    # one packed f32 const tensor: [ident(128) | cpk0|cpk1|cpk2 (39 each) | bqr0..2]
    cpack_d = nc.dram_tensor("cpack", [128, 128 + 3 * (7 + G) + 5], F32,
                             kind="ExternalInput")
    gmapt_d = nc.dram_tensor("gmapt", [G, C], F32, kind="ExternalInput")
    ones77_d = nc.dram_tensor("ones77", [L, L], F32R, kind="ExternalInput")
    identr_d = nc.dram_tensor("identr", [128, 128], F32R, kind="ExternalInput")
    out_d = nc.dram_tensor("out", [C, N], mybir.dt.bfloat16, kind="ExternalOutput")

    with tile.TileContext(nc) as tc:
        _emit(nc, tc, x_d, ctx_d, wqo_d, wkvt_d, cpack_d,
              gmapt_d, ones77_d, identr_d, out_d, reps)
    nc.compile()
    return nc


def _emit(nc, tc, x_d, ctx_d, wqo_d, wkvt_d, cpack_d,
          gmapt_d, ones77_d, identr_d, out_d, reps=1):
    from contextlib import ExitStack

    with ExitStack() as ctx_stack:
        const = ctx_stack.enter_context(tc.tile_pool(name="const", bufs=1))
        xpool = ctx_stack.enter_context(tc.tile_pool(name="xbuf", bufs=1))
        kvres = ctx_stack.enter_context(tc.tile_pool(name="kvres", bufs=1))
        loop = ctx_stack.enter_context(tc.tile_pool(name="loop", bufs=2))
        stats = ctx_stack.enter_context(tc.tile_pool(name="stats", bufs=1))

        psum = ctx_stack.enter_context(tc.tile_pool(name="psum", bufs=8, space="PSUM"))

        def ps_tile(p, f):
            return psum.tile([p, f], F32, tag="mm", name="mm")

        # ---------------- constants ----------------
        # DMA issue order is tuned for ramp time: ident + packed consts first
        # (prologue-critical), ctx/wkv/wqn/wot inside the prologue, then
        # ones77, then the 18 big x slices, then gmapt (needed only at stats).
        NCP = 7 + G
        cpack_sb = const.tile([128, 128 + 3 * NCP + 5], F32, tag="cpack", name="cpack")
        gmt_sb = const.tile([G, C], F32, tag="gmt", name="gmt")
        ones_sb = const.tile([L, L], F32R, tag="ones77", name="ones77")
        id_sb = cpack_sb[:, 0:128]
        cpk_sb = [cpack_sb[0:cs, 128 + i * NCP:128 + (i + 1) * NCP]
                  for i, (c0, cs) in enumerate(CCH)]
        b6_sb = cpk_sb                       # cols 0..5 = bq*s|bk|bv|bo|gamma|beta
        gm_sb = [t[:, 7:7 + G] for t in cpk_sb]
        bqr_sb = [cpack_sb[0:cs, 128 + 3 * NCP + i:128 + 3 * NCP + i + 1]
                  for i, (c0, cs) in enumerate(CCH)]
        cmagic_sb = cpack_sb[0:G, 128 + 3 * NCP + 3:128 + 3 * NCP + 4].bitcast(mybir.dt.int32)
        c15_sb = cpack_sb[0:G, 128 + 3 * NCP + 4:128 + 3 * NCP + 5]
        idr_sb = const.tile([128, 128], F32R, tag="identr", name="identr")
        nc.sync.dma_start(out=cpack_sb, in_=cpack_d.ap())
        nc.sync.dma_start(out=idr_sb, in_=identr_d.ap())

        # x resident chunks (loaded below, after the prologue's DMAs are queued)
        xb = [xpool.tile([cs, N], F32R, tag=f"xb{i}", name=f"xb{i}")
              for i, (c0, cs) in enumerate(CCH)]

        def xv(m, sl):
            return xb[m][:, sl]

        # ---------------- prologue: k, v -> k' (kq), v2t, blk ----------------
        kq_sb = [kvres.tile([cs, L], F32R, tag=f"kq{i}", name=f"kq{i}") for i, (c0, cs) in enumerate(CCH)]
        v2t_sb = kvres.tile([L, C], F32R, tag="v2t", name="v2t")
        blk_sb = kvres.tile([L, 1], F32, tag="blk", name="blk")

        with tc.tile_pool(name="prolA", bufs=1) as prolA:
            # k/v and the fused-projection weights span both prologue phases
            k_sb = [prolA.tile([cs, L], F32, tag=f"k{i}", name=f"k{i}")
                    for i, (c0, cs) in enumerate(CCH)]
            v_sb = [prolA.tile([cs, L], F32, tag=f"v{i}", name=f"v{i}")
                    for i, (c0, cs) in enumerate(CCH)]
            wqo_sb = [prolA.tile([cs, 2 * C], F32, tag=f"wqo{i}", name=f"wqo{i}")
                      for i, (c0, cs) in enumerate(CCH)]
            wqn_sb = [t[:, 0:C] for t in wqo_sb]
            wot_sb = [t[:, C:2 * C] for t in wqo_sb]

            with tc.tile_pool(name="prolB", bufs=1) as prolB:
                kv_ps = [ps_tile(128, L) for _ in range(6)]
                cj_half = []
                for h in range(2):
                    # double-buffered and issued on the idle GpSimd queue so
                    # neither half ever stalls the SP DMA queue
                    ch = prolB.tile([L, CTX // 2], F32, tag="cj_in", name="cj_in", bufs=2)
                    nc.gpsimd.dma_start(out=ch, in_=ctx_d.ap()[:, h * 384:(h + 1) * 384])
                    cj_half.append(ch)
                for j in range(6):
                    tp = ps_tile(128, L)
                    src_h = cj_half[j // 3][:, (j % 3) * 128:(j % 3 + 1) * 128]
                    nc.tensor.transpose(tp, src_h, id_sb[0:L, 0:L])
                    cj = prolB.tile([128, L], F32, tag="ctxt", name="ctxt", bufs=2)
                    nc.scalar.activation(cj, tp, AF.Copy)
                    wkv_j = prolB.tile([128, KV_COLS], F32, tag="wkv", name="wkv", bufs=2)
                    nc.sync.dma_start(out=wkv_j, in_=wkvt_d.ap()[j * 128:(j + 1) * 128, :])
                    for m in range(6):
                        nc.tensor.matmul(kv_ps[m][0:KV_SZ[m], :],
                                         wkv_j[:, KV_OFF[m]:KV_OFF[m] + KV_SZ[m]],
                                         cj, start=(j == 0), stop=(j == 5))
                for i, (c0, cs) in enumerate(CCH):
                    # ACT queue: skips past the SP queue, which creeps in
                    # lock-step with the kv loop on wkv buffer-reuse waits
                    nc.scalar.dma_start(out=wqo_sb[i], in_=wqo_d.ap()[c0:c0 + cs, :])
                for m in range(6):
                    if m < 3:
                        c0, cs = CCH[m]
                        nc.scalar.activation(k_sb[m], kv_ps[m][0:cs, :], AF.Identity,
                                             bias=b6_sb[m][:, BK:BK + 1])
                    else:
                        c0, cs = CCH[m - 3]
                        nc.scalar.activation(v_sb[m - 3], kv_ps[m][0:cs, :], AF.Identity,
                                             bias=b6_sb[m - 3][:, BV:BV + 1])

            # kq[m] = sum_o (Wq*s)[o, m-chunk]^T k[o, :]   -> [ms, 77]
            for m, (m0, ms) in enumerate(CCH):
                kp = ps_tile(ms, L)
                for ki in range(3):
                    nc.tensor.matmul(kp, wqn_sb[ki][:, m0:m0 + ms], k_sb[ki],
                                     start=(ki == 0), stop=(ki == 2))
                nc.scalar.activation(kq_sb[m], kp, AF.Copy)
            # v2t = (Wo v)^T = v^T Wo^T  -> [77, 320]
            vp = ps_tile(L, C)
            for ki in range(3):
                nc.tensor.matmul(vp, v_sb[ki], wot_sb[ki],
                                 start=(ki == 0), stop=(ki == 2))
            nc.scalar.activation(v2t_sb, vp, AF.Copy)
            # blk = k^T (bq*s)  -> [77, 1]
            bp = ps_tile(L, 1)
            for ki in range(3):
                nc.tensor.matmul(bp, k_sb[ki], bqr_sb[ki],
                                 start=(ki == 0), stop=(ki == 2))
            nc.scalar.activation(blk_sb, bp, AF.Copy)

        # one-time consts for the rep body
        nc.sync.dma_start(out=ones_sb, in_=ones77_d.ap())
        nc.sync.dma_start(out=gmt_sb, in_=gmapt_d.ap())
        sm = ctx_stack.enter_context(tc.tile_pool(name="sm", bufs=1))
        for _rep in range(reps):
            s0 = 0
            for w in X_WIDTHS:
                ssl = slice(s0, s0 + w)
                for i, (c0, cs) in enumerate(CCH):
                    nc.sync.dma_start(out=xv(i, ssl), in_=x_d.ap()[c0:c0 + cs, ssl])
                s0 += w

            # ---------------- main loop (software-pipelined, 4 stages) ---------
            # Per tile t: FRONT = scores+exp (PE/ACT), MID = recip+mul
            # (DVE/Pool) one tile behind, TAIL = o-matmul+residual+copy
            # (PE/ACT) two tiles behind, BNSTATS (DVE) three tiles behind;
            # colsum (PE) is emitted last in each iteration.  The stage skew
            # keeps every engine's in-order queue fed with ready instructions.
            st_sb = [stats.tile([cs, NT, 6], F32, tag=f"st{i}", name=f"st{i}") for i, (c0, cs) in enumerate(CCH)]
            u_t, cb_t, rb_t = {}, {}, {}

            def front(it):
                nsl = slice(it * N_TILE, (it + 1) * N_TILE)
                # scores [77, n] = sum_k kq[k].T @ x[k]  (q-proj folded into kq)
                sp = ps_tile(L, N_TILE)
                for ki in range(3):
                    nc.tensor.matmul(sp, kq_sb[ki], xv(ki, nsl),
                                     start=(ki == 0), stop=(ki == 2))
                u = loop.tile([L, N_TILE], F32R, tag="u", name="u", bufs=3)
                nc.scalar.activation(u, sp, AF.Exp, bias=blk_sb)
                u_t[it] = u

            def colsum(it):
                # colsum of u broadcast to 77 partitions via all-ones lhsT
                cb = ps_tile(L, N_TILE)
                nc.tensor.matmul(cb, ones_sb, u_t[it], start=True, stop=True)
                cb_t[it] = cb

            def mid(it):
                rb = loop.tile([L, N_TILE], F32, tag="rb", name="rb", bufs=2)
                nc.vector.reciprocal(out=rb, in_=cb_t.pop(it))
                nc.gpsimd.tensor_mul(u_t[it], u_t[it].bitcast(F32), rb)

            def tail(it):
                nsl = slice(it * N_TILE, (it + 1) * N_TILE)
                u = u_t.pop(it)
                # xr[m] = v2t[:, m-chunk].T @ u + x[m]  (residual accumulated
                # on PE via identity matmul); ACT copies PSUM->SBUF in place.
                ops = []
                for m, (m0, ms) in enumerate(CCH):
                    op = ps_tile(ms, N_TILE)
                    nc.tensor.matmul(op, v2t_sb[:, m0:m0 + ms], u,
                                     start=True, stop=False)
                    nc.tensor.matmul(op, idr_sb[0:ms, 0:ms], xv(m, nsl),
                                     start=False, stop=True)
                    ops.append(op)
                for m, (m0, ms) in enumerate(CCH):
                    nc.scalar.activation(xb[m][:, nsl], ops[m], AF.Copy)

            def bnstats(it):
                nsl = slice(it * N_TILE, (it + 1) * N_TILE)
                for m, (m0, ms) in enumerate(CCH):
                    nc.vector.bn_stats(out=st_sb[m][:, it, :],
                                       in_=xv(m, nsl).bitcast(F32))

            for it in range(NT + 3):
                if it < NT:
                    front(it)
                if 0 <= it - 2 < NT:
                    tail(it - 2)
                if 0 <= it - 1 < NT:
                    mid(it - 1)
                if it < NT:
                    colsum(it)
                if 0 <= it - 3 < NT:
                    bnstats(it - 3)

            # ---------------- groupnorm stats ----------------
            st3 = []
            for i, (c0, cs) in enumerate(CCH):
                mv = sm.tile([cs, 2], F32, tag=f"mv{i}", name=f"mv{i}")
                nc.vector.bn_aggr(out=mv, in_=st_sb[i])
                s3 = sm.tile([cs, 3], F32, tag=f"s3{i}", name=f"s3{i}")
                # mean' = mean + bo ; var ; mean'^2
                nc.vector.tensor_add(s3[:, 0:1], mv[:, 0:1], b6_sb[i][:, BO:BO + 1])
                nc.vector.tensor_copy(s3[:, 1:2], mv[:, 1:2])
                nc.vector.tensor_mul(s3[:, 2:3], s3[:, 0:1], s3[:, 0:1])
                st3.append(s3)
            gp = ps_tile(G, 3)
            for i in range(3):
                nc.tensor.matmul(gp, gm_sb[i], st3[i], start=(i == 0), stop=(i == 2))
            # group stats: mu = s_mean/GS ; var = (s_var + s_mean2)/GS - mu^2
            gs = sm.tile([G, 3], F32, tag="gs", name="gs")
            nc.scalar.activation(gs, gp, AF.Copy)
            mu = sm.tile([G, 1], F32, tag="mu", name="mu")
            nc.scalar.activation(mu, gs[:, 0:1], AF.Copy, scale=1.0 / GS)
            tvar = sm.tile([G, 1], F32, tag="tvar", name="tvar")
            nc.vector.tensor_add(tvar, gs[:, 1:2], gs[:, 2:3])
            mu2 = sm.tile([G, 1], F32, tag="mu2", name="mu2")
            nc.vector.tensor_mul(mu2, mu, mu)
            var = sm.tile([G, 1], F32, tag="var", name="var")
            nc.vector.scalar_tensor_tensor(
                out=var, in0=tvar, scalar=1.0 / GS, in1=mu2,
                op0=ALU.mult, op1=ALU.subtract)
            # rstd = 1/sqrt(var + eps) on DVE via the bit-trick + 2 Newton
            # iterations (avoids swapping the ACT table away from Exp)
            I32 = mybir.dt.int32
            vpe = sm.tile([G, 1], F32, tag="vpe", name="vpe")
            nc.vector.tensor_scalar_add(vpe, var, EPS)
            y0i = sm.tile([G, 1], I32, tag="y0i", name="y0i")
            nc.vector.tensor_scalar(out=y0i, in0=vpe.bitcast(I32), scalar1=1,
                                    scalar2=None, op0=ALU.arith_shift_right)
            nc.vector.tensor_sub(y0i, cmagic_sb, y0i)
            rstd = sm.tile([G, 1], F32, tag="rstd", name="rstd")
            nc.vector.tensor_copy(rstd, y0i.bitcast(F32))
            yt = sm.tile([G, 1], F32, tag="yt", name="yt")
            for _nw in range(2):
                nc.vector.tensor_mul(yt, rstd, rstd)
                nc.vector.tensor_mul(yt, yt, vpe)
                nc.vector.scalar_tensor_tensor(
                    out=yt, in0=yt, scalar=-0.5, in1=c15_sb,
                    op0=ALU.mult, op1=ALU.add)
                nc.vector.tensor_mul(rstd, rstd, yt)
            mr = sm.tile([G, 2], F32, tag="mr", name="mr")
            nc.vector.tensor_copy(mr[:, 0:1], mu)
            nc.vector.tensor_copy(mr[:, 1:2], rstd)
            # broadcast back per channel: [cs, 2] = gmapT[m].T @ mr
            ab = []
            for m, (m0, ms) in enumerate(CCH):
                bp = ps_tile(ms, 2)
                nc.tensor.matmul(bp, gmt_sb[:, m0:m0 + ms], mr, start=True, stop=True)
                a_m = sm.tile([ms, 1], F32, tag=f"a{m}", name=f"a{m}")
                nc.vector.tensor_mul(a_m, bp[:, 1:2], b6_sb[m][:, GA:GA + 1])
                # b = beta + a*(bo - mu)
                t1 = sm.tile([ms, 1], F32, tag=f"t1{m}", name=f"t1{m}")
                nc.vector.tensor_sub(t1, b6_sb[m][:, BO:BO + 1], bp[:, 0:1])
                t2 = sm.tile([ms, 1], F32, tag=f"t2{m}", name=f"t2{m}")
                nc.vector.tensor_mul(t2, t1, a_m)
                b_m = sm.tile([ms, 1], F32, tag=f"b{m}", name=f"b{m}")
                nc.vector.tensor_add(b_m, b6_sb[m][:, BE:BE + 1], t2)
                ab.append((a_m, b_m))

            # ---------------- pass 2: swish + store (bf16 out halves DMA) -------
            for s0 in range(0, N, O_SLICE):
                ssl = slice(s0, s0 + O_SLICE)
                for m, (m0, ms) in enumerate(CCH):
                    a_m, b_m = ab[m]
                    xmv = xv(m, ssl).bitcast(F32)
                    ob = loop.tile([ms, O_SLICE], BF16, tag="ob", name="ob", bufs=3)
                    if USE_SILU:
                        nc.scalar.activation(ob, xmv, AF.Silu, bias=b_m, scale=a_m)
                    else:
                        # sim fallback: xn*sigmoid(xn) with xn = a*x + b, via
                        #   sig = sigmoid(a*x+b); t = (x*a)*sig; out = (sig*b) + t
                        sgm = loop.tile([ms, O_SLICE], F32, tag="sg", name="sg", bufs=1)
                        nc.scalar.activation(sgm, xmv, AF.Sigmoid, bias=b_m, scale=a_m)
                        nc.vector.scalar_tensor_tensor(
                            out=xmv, in0=xmv, scalar=a_m, in1=sgm,
                            op0=ALU.mult, op1=ALU.mult)
                        nc.vector.scalar_tensor_tensor(
                            out=xmv, in0=sgm, scalar=b_m, in1=xmv,
                            op0=ALU.mult, op1=ALU.add)
                        nc.vector.tensor_copy(ob, xmv)
                    nc.sync.dma_start(out=out_d.ap()[m0:m0 + ms, ssl], in_=ob)


_NC_CACHE = None


def _get_nc():
    global _NC_CACHE
    if _NC_CACHE is None:
        _NC_CACHE = _build()
    return _NC_CACHE


def _host_consts(Wq, bq, Wk, bk, Wv, bv, Wo, bo, gamma, beta):
    s = float(C) ** -0.5
    wqo = np.concatenate([(Wq * s).astype(np.float32),
                          Wo.T.astype(np.float32)], axis=1)
    wqo = np.ascontiguousarray(wqo)
    wkvt = np.concatenate([Wk.T.astype(np.float32),
                           Wv.T.astype(np.float32)], axis=1)  # [CTX, 640]
    bias6 = np.stack([bq * s, bk, bv, bo, gamma, beta], axis=1).astype(np.float32)
    gmap = np.zeros((C, G), np.float32)
    gmap[np.arange(C), np.arange(C) // GS] = 1.0
    # per-channel consts [bias6 | bq*s | gmap], packed per chunk into cpack
    cpk = np.concatenate(
        [bias6, (bq * s).astype(np.float32).reshape(C, 1), gmap], axis=1)
    NCP = 7 + G
    cpack = np.zeros((128, 128 + 3 * NCP + 5), np.float32)
    cpack[:, 0:128] = np.eye(128, dtype=np.float32)
    for i, (c0, cs) in enumerate(CCH):
        cpack[0:cs, 128 + i * NCP:128 + (i + 1) * NCP] = cpk[c0:c0 + cs, :]
        cpack[0:cs, 128 + 3 * NCP + i] = (bq[c0:c0 + cs] * s).astype(np.float32)
    cpack[:, 128 + 3 * NCP + 3] = np.float32(
        np.frombuffer(np.uint32(0x5F3759DF).tobytes(), np.float32)[0])
    cpack[:, 128 + 3 * NCP + 4] = 1.5
    gmapt = np.ascontiguousarray(gmap.T)
    ones77 = np.ones((L, L), np.float32)
    ident = np.eye(128, dtype=np.float32)
    return dict(wqo=wqo, wkvt=wkvt, cpack=np.ascontiguousarray(cpack),
                gmapt=gmapt, ones77=ones77, identr=ident)


def kernel(x, context, Wq, bq, Wk, bk, Wv, bv, Wo, bo, gamma, beta,
           _return_results=False, _trace=False):
    x = np.asarray(x, np.float32)
    context = np.asarray(context, np.float32)
    consts = _host_consts(np.asarray(Wq, np.float32), np.asarray(bq, np.float32),
                          np.asarray(Wk, np.float32), np.asarray(bk, np.float32),
                          np.asarray(Wv, np.float32), np.asarray(bv, np.float32),
                          np.asarray(Wo, np.float32), np.asarray(bo, np.float32),
                          np.asarray(gamma, np.float32), np.asarray(beta, np.float32))
    nc = _get_nc()
    in_maps = []
    for b in range(B):
        m = dict(consts)
        m["x"] = np.ascontiguousarray(x[b].reshape(C, N))
        m["ctx"] = np.ascontiguousarray(context[b])
        in_maps.append(m)
    res = run_bass_kernel_spmd(nc, in_maps, core_ids=list(range(B)), trace=_trace)
    out = np.stack([np.asarray(res.results[b]["out"], dtype=np.float32)
                    .reshape(C, D, D, D) for b in range(B)])
    if _return_results:
        return out, res
    return out

